# revision 72
# baseline (speedup 1.0000x reference)
"""Single transformer block on 8 NeuronCores.

Sharding: core c handles batch b=c//2, sequence half c%2 (T=1024 tokens,
interleaved in stripes of BS=256 for causal load balance). All token-wise ops
(LN, QKV, c_proj, MLP) are purely local; attention needs the full sequence of
K/V per batch, obtained with a pairwise fp8 AllGather between cores
{2b, 2b+1}.

Dtype strategy (validated numerically, rel_l2 targets << 2e-2):
  - K, V, LN1 output h^T, W_attn: fp8e4m3 (W_attn host-prescaled by 50x so
    weights sit in the fp8 normal range; descaled at PSUM eviction)
  - QKV matmuls run fp8 DoubleRow (two 128-deep contraction subtiles per
    instruction)
  - everything else (q, P=exp(S), attention out, c_proj / fc / mproj weights
    and activations): bf16 inputs, f32 accumulation
  - residual stream x, x2: f32

Layout strategy (per core):
  - residual stream x: token-major [128t x D] SBUF tiles
  - h^T, m^T: feature-major via PE transpose; LN's affine (w, b) is folded
    into the per-partition scale/bias of the transpose eviction
  - scores computed transposed S^T[k, q]; softmax denominator comes free
    from an appended ones-column in V during the AV matmul; normalization
    uses a Pool partition_broadcast of the reciprocal row
  - causal mask applied as a 0/1 multiply on P=exp(S) boundary chunks only
  - exp is evaluated on chunk PAIRS ([128, 2, w] PSUM tiles) to halve the
    fixed per-call activation cost
  - biases: qkv biases via ACT eviction bias; b_v folded into an effective
    c_proj bias host-side (softmax rows sum to 1); c_proj/mproj biases added
    as rank-1 matmuls into PSUM; fc bias via gelu eviction bias
"""

import math
from contextlib import ExitStack

import numpy as np
import ml_dtypes

import concourse.bacc as bacc
import concourse.bass as bass
import concourse.mybir as mybir
import concourse.tile as tile
from concourse.masks import make_identity

F32 = mybir.dt.float32
F32R = mybir.dt.float32r
BF16 = mybir.dt.bfloat16
F8 = mybir.dt.float8e4
AF = mybir.ActivationFunctionType
ALU = mybir.AluOpType
DR = mybir.MatmulPerfMode.DoubleRow

EPS = 1e-5
WS = 50.0  # host-side W_attn scale so fp8 weights stay in normal range


class Cfg:
    def __init__(self, B=4, S=2048, D=1024, H=16, F=4096, n_cores=8, bs=256):
        self.B, self.S, self.D, self.H, self.F = B, S, D, H, F
        self.n_cores = n_cores
        assert n_cores == 2 * B
        self.HD = D // H
        assert self.HD == 64
        self.T = S // 2            # tokens per core
        self.TB = self.T // 128    # token 128-blocks
        self.DC = D // 128         # contraction chunks over D
        self.QF = min(512, self.T)  # q free-dim tile
        self.QH = self.T // self.QF
        self.KC = S // 128         # key 128-chunks over full sequence
        self.VF = min(512, D)      # out-feature tile for token-major outs
        self.FH = D // self.VF
        self.GB = F // 128         # MLP hidden 128-blocks
        self.HPB = 128 // self.HD  # heads per 128-feature block (=2)
        self.BS = min(bs, self.T)  # stripe block (q-slot) size
        self.SLOTS = self.T // self.BS
        self.KCH = self.KC // 2    # AG chunks per parity block
        self.CPB = self.BS // 128  # 128-chunks per stripe block


def chunk_absblk(c, kc):
    # absolute stripe-block index covered by AG chunk kc
    parity = kc // c.KCH
    loc = kc % c.KCH
    return 2 * ((loc * 128) // c.BS) + parity


def pairup(lst):
    """[(a,b), (c,d), ...] consecutive pairs; assumes even length."""
    assert len(lst) % 2 == 0
    return [(lst[i], lst[i + 1]) for i in range(0, len(lst), 2)]


def build(cfg: Cfg):
    c = cfg
    nc = bacc.Bacc(None, target_bir_lowering=False)

    # ---------------- I/O ----------------
    x_in = nc.dram_tensor("x", [c.T, c.D], F32, kind="ExternalInput")
    w_attn = nc.dram_tensor("w_attn", [c.D, 3 * c.D], F8, kind="ExternalInput")
    w_cproj = nc.dram_tensor("w_cproj", [c.D, c.D], BF16, kind="ExternalInput")
    # fc / mproj weights as compensated fp8 pairs (W*WS = W8 + dW8): the
    # GEMMs run 3 DoubleRow passes (W8@h8 + W8@dh8 + dW8@h8) at 0.75x the
    # bf16 PE cost and better-than-bf16 accuracy.
    w_fc8 = nc.dram_tensor("w_fc8", [c.D, c.F], F8, kind="ExternalInput")
    dw_fc8 = nc.dram_tensor("dw_fc8", [c.D, c.F], F8, kind="ExternalInput")
    w_mp8 = nc.dram_tensor("w_mp8", [c.F, c.D], F8, kind="ExternalInput")
    dw_mp8 = nc.dram_tensor("dw_mp8", [c.F, c.D], F8, kind="ExternalInput")
    ln1wc_in = nc.dram_tensor("ln1wc", [128, c.DC], F32, kind="ExternalInput")
    ln1bc_in = nc.dram_tensor("ln1bc", [128, c.DC], F32, kind="ExternalInput")
    ln2wc_in = nc.dram_tensor("ln2wc", [128, c.DC], F32, kind="ExternalInput")
    ln2bc_in = nc.dram_tensor("ln2bc", [128, c.DC], F32, kind="ExternalInput")
    battn_qk_in = nc.dram_tensor("battn_qk", [128, 2 * c.DC], F32,
                                 kind="ExternalInput")
    bcp_in = nc.dram_tensor("bcp", [1, c.D], BF16, kind="ExternalInput")
    bmp_in = nc.dram_tensor("bmp", [1, c.D], BF16, kind="ExternalInput")
    bfc_in = nc.dram_tensor("bfc", [128, c.GB], F32, kind="ExternalInput")
    qidx_in = nc.dram_tensor("qidx", [1, c.T], F32, kind="ExternalInput")
    kofs_in = nc.dram_tensor("kofs", [128, c.KC], F32, kind="ExternalInput")
    y_out = nc.dram_tensor("y", [c.T, c.D], F32, kind="ExternalOutput")

    pairs = [[2 * b, 2 * b + 1] for b in range(c.B)]

    def bcast(dram, p=128):
        # partition-broadcast DMA source: read row 0 for every partition
        return bass.AP(tensor=dram, offset=0, ap=[[0, p], [1, dram.shape[1]]])

    with tile.TileContext(nc) as tc, ExitStack() as es:
        dpool = es.enter_context(tc.tile_pool(name="dram", bufs=1, space="DRAM"))
        gconst = es.enter_context(tc.tile_pool(name="gconst", bufs=1))

        # DRAM bounce buffers for the pairwise AllGathers (fp8), split into
        # head-halves so each AG launches as soon as its half is produced
        # and attention unblocks incrementally.
        HB = c.D // 2  # feature rows per head-half
        kb_loc = [dpool.tile([HB, c.T], F8, name=f"kb_loc{h}")
                  for h in range(2)]
        kb_full = [dpool.tile([2 * HB, c.T], F8, name=f"kb_full{h}")
                   for h in range(2)]
        vb_loc = [dpool.tile([c.T, HB], F8, name=f"vb_loc{h}")
                  for h in range(2)]
        vb_full = [dpool.tile([2 * c.T, HB], F8, name=f"vb_full{h}")
                   for h in range(2)]

        # ---------------- global constants ----------------
        ident_bf = gconst.tile([128, 128], BF16)
        make_identity(nc, ident_bf[:])
        eps_t = gconst.tile([128, 1], F32)
        nc.vector.memset(eps_t[:], EPS)
        ones1_bf = gconst.tile([1, 128], BF16)
        nc.vector.memset(ones1_bf[:], 1.0)

        def layernorm_t(src_tiles, wcol, bcol, out_tile, out_tag,
                        dout_tile=None):
            """token-major LN over free axis + transpose to feature-major.

            Writes DC slices of out_tile [128, DC, T]; the LN affine (w, b)
            is applied per-partition at the transpose eviction. If dout_tile
            is given, also writes the fp8 quantization residual
            (exact - out) for compensated-fp8 GEMMs."""
            with (
                tc.tile_pool(name=f"ln_{out_tag}", bufs=3) as lnp,
                tc.tile_pool(name=f"ln2_{out_tag}", bufs=4) as lnp2,
                tc.tile_pool(name=f"ps_tr_{out_tag}", bufs=4,
                             space="PSUM") as ps_tr,
            ):
                for tb in range(c.TB):
                    src = src_tiles[tb]
                    nsg = c.D // 512 if c.D % 512 == 0 else 1
                    sgw = c.D // nsg
                    st = lnp.tile([128, nsg, 6], F32, tag="st")
                    for sg in range(nsg):
                        nc.vector.bn_stats(
                            out=st[:, sg, :],
                            in_=src[:, sg * sgw:(sg + 1) * sgw])
                    mv = lnp.tile([128, 2], F32, tag="mv")
                    nc.vector.bn_aggr(out=mv[:], in_=st[:])
                    sd = lnp.tile([128, 1], F32, tag="sd")
                    nc.scalar.activation(sd[:], mv[:, 1:2], AF.Sqrt,
                                         bias=eps_t[:, 0:1])
                    rs = lnp.tile([128, 1], F32, tag="rs")
                    nc.vector.reciprocal(rs[:], sd[:])
                    ht_ = lnp.tile([128, c.D], BF16, tag="h")
                    # normalize split DVE/Pool, sized by their relative
                    # throughput, to shorten the per-tile critical chain
                    hD = 640
                    nc.vector.tensor_scalar(
                        out=ht_[:, 0:hD], in0=src[:, 0:hD],
                        scalar1=mv[:, 0:1], scalar2=rs[:, 0:1],
                        op0=ALU.subtract, op1=ALU.mult)
                    nc.gpsimd.tensor_scalar(
                        out=ht_[:, hD:c.D], in0=src[:, hD:c.D],
                        scalar1=mv[:, 0:1], scalar2=rs[:, 0:1],
                        op0=ALU.subtract, op1=ALU.mult)
                    for i in range(c.DC):
                        pt = ps_tr.tile([128, 128], BF16, tag="tr")
                        nc.tensor.transpose(
                            pt[:], ht_[:, i * 128:(i + 1) * 128], ident_bf[:])
                        # ACT evict with the LN affine folded in as
                        # per-partition scale/bias (Pool cannot read PSUM)
                        tsl = slice(tb * 128, (tb + 1) * 128)
                        nc.scalar.activation(
                            out_tile[:, i, tsl], pt[:],
                            AF.Identity, bias=bcol[:, i:i + 1],
                            scale=wcol[:, i:i + 1])
                        if dout_tile is not None:
                            tmp = lnp2.tile([128, 128], BF16, tag="tmp")
                            nc.vector.tensor_scalar(
                                out=tmp[:], in0=pt[:],
                                scalar1=wcol[:, i:i + 1],
                                scalar2=bcol[:, i:i + 1],
                                op0=ALU.mult, op1=ALU.add)
                            nc.vector.tensor_tensor(
                                out=dout_tile[:, i, tsl], in0=tmp[:],
                                in1=out_tile[:, i, tsl],
                                op=ALU.subtract)

        # ================= phase A: LN1 + QKV =================
        es_x = ExitStack()
        xpool = es_x.enter_context(tc.tile_pool(name="xpool", bufs=1, side="left"))
        xt = []
        for tb in range(c.TB):
            t = xpool.tile([128, c.D], F32, tag=f"x{tb}", name=f"x{tb}")
            # alternate DMA queues and fetch halves so LN stats (which work
            # on 512-wide subgroups) start as early as possible
            eng = nc.sync if tb % 2 == 0 else nc.gpsimd
            for hx in range(2):
                csl = slice(hx * 512, (hx + 1) * 512)
                eng.dma_start(out=t[:, csl],
                              in_=x_in[tb * 128:(tb + 1) * 128, csl])
            xt.append(t)

        es_qt = ExitStack()
        qtpool = es_qt.enter_context(tc.tile_pool(name="qtpool", bufs=1, side="right"))
        qtp = []
        for j in range(c.H // c.HPB):
            qtp.append(qtpool.tile([128, c.T], BF16, tag=f"qt{j}",
                                   name=f"qt{j}"))

        with (
            tc.tile_pool(name="aconst", bufs=1) as aconst,
            tc.tile_pool(name="htp", bufs=1) as htpool,
        ):
            ln1wc = aconst.tile([128, c.DC], F32)
            ln1bc = aconst.tile([128, c.DC], F32)
            for t, d in [(ln1wc, ln1wc_in), (ln1bc, ln1bc_in)]:
                nc.sync.dma_start(out=t[:], in_=d[:, :])
            battn_qk = aconst.tile([128, 2 * c.DC], F32)
            nc.sync.dma_start(out=battn_qk[:], in_=battn_qk_in[:, :])

            NP = c.DC // 2  # DoubleRow contraction pairs

            with (
                tc.tile_pool(name="wa", bufs=3) as wap,
                tc.tile_pool(name="kout", bufs=3) as kop,
                tc.tile_pool(name="ps_mm", bufs=4, space="PSUM") as psmm,
            ):
                # allocate + fetch the QKV weights BEFORE the LN pools so
                # their SBUF space doesn't alias LN transients (which would
                # make the DMA wait for LN1 to release buffers)
                wk_all = wap.tile([128, c.DC, c.D], F8, tag="wa",
                                  name="wk_all")
                nc.sync.dma_start(
                    out=wk_all[:],
                    in_=w_attn[:, c.D:2 * c.D].rearrange(
                        "(i p) f -> p i f", p=128))
                wv_all = wap.tile([128, c.DC, c.D], F8, tag="wa",
                                  name="wv_all")
                nc.sync.dma_start(
                    out=wv_all[:],
                    in_=w_attn[:, 2 * c.D:3 * c.D].rearrange(
                        "(i p) f -> p i f", p=128))

                ht = htpool.tile([128, c.DC, c.T], F8, name="ht_all")
                layernorm_t(xt, ln1wc, ln1bc, ht, "ht")

                # ---- K and V passes, interleaved by head-half so the
                # collective order is K0, V0, K1, V1 (attention for the
                # first head-half unblocks while the second half transfers)
                MH = c.DC // 2  # m-chunks per head-half
                for kh in range(2):
                    # k^T pass for this head-half (feature-major)
                    for ml in range(MH):
                        m = kh * MH + ml
                        for th in range(c.QH):
                            ps = psmm.tile([128, c.QF], F32, tag="ps")
                            for i in range(NP):
                                nc.tensor.matmul(
                                    ps[:], wk_all[:, 2 * i:2 * i + 2,
                                                  m * 128:(m + 1) * 128],
                                    ht[:, 2 * i:2 * i + 2,
                                       th * c.QF:(th + 1) * c.QF],
                                    start=(i == 0), stop=(i == NP - 1),
                                    perf_mode=DR)
                            ko = kop.tile([128, c.QF], F8, tag="ko")
                            nc.scalar.activation(
                                ko[:], ps[:], AF.Identity,
                                bias=battn_qk[:, c.DC + m:c.DC + m + 1],
                                scale=1.0 / WS)
                            nc.sync.dma_start(
                                out=kb_loc[kh][ml * 128:(ml + 1) * 128,
                                               th * c.QF:(th + 1) * c.QF],
                                in_=ko[:])
                    nc.gpsimd.collective_compute(
                        "AllGather", ALU.bypass, ins=[kb_loc[kh][:]],
                        outs=[kb_full[kh][:]], replica_groups=pairs)
                    # v pass for this head-half (token-major)
                    for tb in range(c.TB):
                        ps = psmm.tile([128, c.VF], F32, tag="ps")
                        for i in range(NP):
                            nc.tensor.matmul(
                                ps[:], ht[:, 2 * i:2 * i + 2,
                                          tb * 128:(tb + 1) * 128],
                                wv_all[:, 2 * i:2 * i + 2,
                                       kh * c.VF:(kh + 1) * c.VF],
                                start=(i == 0), stop=(i == NP - 1),
                                perf_mode=DR)
                        vo = kop.tile([128, c.VF], F8, tag="vo")
                        nc.vector.tensor_scalar(
                            out=vo[:], in0=ps[:], scalar1=1.0 / WS,
                            scalar2=None, op0=ALU.mult)
                        nc.sync.dma_start(
                            out=vb_loc[kh][tb * 128:(tb + 1) * 128, :],
                            in_=vo[:])
                    nc.gpsimd.collective_compute(
                        "AllGather", ALU.bypass, ins=[vb_loc[kh][:]],
                        outs=[vb_full[kh][:]], replica_groups=pairs)

                # ---- q^T pass (feature-major, stays in SBUF) ----
                wq_all = wap.tile([128, c.DC, c.D], F8, tag="wa",
                                  name="wq_all")
                nc.sync.dma_start(
                    out=wq_all[:],
                    in_=w_attn[:, 0:c.D].rearrange(
                        "(i p) f -> p i f", p=128))
                for m in range(c.DC):
                    for th in range(c.QH):
                        ps = psmm.tile([128, c.QF], F32, tag="ps")
                        for i in range(NP):
                            nc.tensor.matmul(
                                ps[:], wq_all[:, 2 * i:2 * i + 2,
                                              m * 128:(m + 1) * 128],
                                ht[:, 2 * i:2 * i + 2,
                                   th * c.QF:(th + 1) * c.QF],
                                start=(i == 0), stop=(i == NP - 1),
                                perf_mode=DR)
                        # scale by 1/sqrt(HD) at eviction (bias pre-scaled)
                        nc.scalar.activation(
                            qtp[m][:, th * c.QF:(th + 1) * c.QF], ps[:],
                            AF.Identity, bias=battn_qk[:, m:m + 1],
                            scale=1.0 / (WS * math.sqrt(c.HD)))

        # ================= phase B: attention =================
        # prefetch c_proj weights during attention (scalar DMA ring)
        es_wc = ExitStack()
        wcp = es_wc.enter_context(tc.tile_pool(name="wc", bufs=1, side="left"))
        wc_all = wcp.tile([128, c.DC, c.D], BF16, tag="wc", name="wc_all")
        nc.scalar.dma_start(
            out=wc_all[:],
            in_=w_cproj[:, :].rearrange("(i p) f -> p i f", p=128))

        es_at = ExitStack()
        atpool = es_at.enter_context(tc.tile_pool(name="atpool", bufs=1, side="left"))
        at = []
        for j in range(c.DC):
            at.append(atpool.tile([128, c.T], BF16, tag=f"at{j}",
                                  name=f"at{j}"))

        with (
            tc.tile_pool(name="bconst", bufs=1) as bconst,
            tc.tile_pool(name="mask", bufs=1) as maskp,
            tc.tile_pool(name="kv", bufs=5) as kvp,
            tc.tile_pool(name="pt5", bufs=56) as ptp5,
            tc.tile_pool(name="pt2", bufs=28) as ptp2,
            tc.tile_pool(name="rec", bufs=4) as recp,
            tc.tile_pool(name="ps_s", bufs=3, space="PSUM") as pss,
            tc.tile_pool(name="ps_o", bufs=2, space="PSUM") as pso,
        ):
            qidx = bconst.tile([128, c.T], F32)
            nc.sync.dma_start(out=qidx[:], in_=bcast(qidx_in))
            kofs = bconst.tile([128, c.KC], F32)
            nc.sync.dma_start(out=kofs[:], in_=kofs_in[:, :])

            # per-slot chunk lists (compile-time causal structure)
            slot_chunks = []
            for sl in range(c.SLOTS):
                cl = [kc for kc in range(c.KC)
                      if chunk_absblk(c, kc) <= 2 * sl + 1]
                slot_chunks.append(cl)

            # group q-slots in pairs: one 512-wide QK/exp per k-chunk
            groups = []
            sl = 0
            while sl < c.SLOTS:
                g = [sl, sl + 1] if sl + 1 < c.SLOTS else [sl]
                groups.append(g)
                sl += len(g)

            # pre-generate boundary masks per (group, chunk) where the chunk
            # may cross the causal diagonal. For "full"-class boundary chunks
            # (absblk <= 2*g0+1) only the LOWER slot of the group can be
            # non-visible (the upper slot's stripes sit strictly after the
            # chunk), so every mask is one slot (BS) wide: full-class masks
            # cover slot g0's columns, diff-class masks cover slot g1's.
            masks = {}
            for gi, g in enumerate(groups):
                for kc in slot_chunks[g[-1]]:
                    ab = chunk_absblk(c, kc)
                    if ab < 2 * g[0]:
                        continue
                    msl_slot = g[0] if ab <= 2 * g[0] + 1 else g[-1]
                    qsl = slice(msl_slot * c.BS, (msl_slot + 1) * c.BS)
                    mk = maskp.tile([128, c.BS], BF16,
                                    tag=f"mk{gi}_{kc}",
                                    name=f"mk{gi}_{kc}")
                    nc.vector.tensor_scalar(
                        out=mk[:], in0=qidx[:, qsl],
                        scalar1=kofs[:, kc:kc + 1], scalar2=None,
                        op0=ALU.is_ge)
                    masks[(gi, kc)] = mk

            for jj in range(c.H // c.HPB):
                kh = jj // 4        # head-half buffer index
                jl = jj % 4
                ktp = kvp.tile([128, c.S], F8, tag="ktp")
                for hp in range(c.HPB):
                    hl = 2 * jl + hp  # head within the half
                    psl = slice(hp * 64, hp * 64 + 64)
                    nc.sync.dma_start(
                        out=ktp[psl, 0:c.T],
                        in_=kb_full[kh][64 * hl:64 * hl + 64, :])
                    nc.sync.dma_start(
                        out=ktp[psl, c.T:c.S],
                        in_=kb_full[kh][HB + 64 * hl:HB + 64 * hl + 64, :])
                # V for both heads of the pair, with an appended ones column
                # per head. Chunk row padded to 144 so the DoubleRow
                # Ldweights outer step is 16B-aligned (head slots at 0, 72).
                vt = kvp.tile([128, c.KC, 144], F8, tag="vt")
                vt4 = vt[:, :, :].rearrange("p kc (h f) -> p kc h f", f=72)
                for hp in range(c.HPB):
                    fb = 128 * jl + 64 * hp
                    nc.sync.dma_start(
                        out=vt[:, :, hp * 72:hp * 72 + 64],
                        in_=vb_full[kh][:, fb:fb + 64].rearrange(
                            "(kc p) f -> p kc f", p=128))
                nc.gpsimd.memset(vt4[:, :, :, 64:65], 1.0)

                for hp in range(c.HPB):
                    base = hp * 64
                    for gi, g in enumerate(groups):
                        gw = len(g) * c.BS
                        gq = slice(g[0] * c.BS, g[0] * c.BS + gw)
                        rhs_q = qtp[jj][base:base + 64, gq]
                        cl_all = slot_chunks[g[-1]]
                        full = [kc for kc in cl_all
                                if not (len(g) == 2 and
                                        chunk_absblk(c, kc) > 2 * g[0] + 1)]
                        diff = [kc for kc in cl_all if kc not in full]
                        pt_of = {}
                        nmask = 0
                        for plist, w, dtag in ((full, gw, False),
                                               (diff, c.BS, True)):
                            rq = (qtp[jj][base:base + 64,
                                          g[1] * c.BS:(g[1] + 1) * c.BS]
                                  if dtag else rhs_q)
                            for kc0, kc1 in pairup(plist):
                                ps = pss.tile([128, 2, gw], F32, tag="s")
                                for j, kc in ((0, kc0), (1, kc1)):
                                    nc.tensor.matmul(
                                        ps[:, j, 0:w],
                                        ktp[base:base + 64,
                                            kc * 128:(kc + 1) * 128],
                                        rq, start=True, stop=True)
                                ptpool = ptp5 if w == gw else ptp2
                                pt = ptpool.tile([128, 2, w], F8,
                                                 tag=f"pt{w}")
                                nc.scalar.activation(pt[:, :, :],
                                                     ps[:, :, 0:w], AF.Exp)
                                for j, kc in ((0, kc0), (1, kc1)):
                                    if (gi, kc) in masks:
                                        mw = masks[(gi, kc)]
                                        # full-class masks only touch the
                                        # lower slot's BS columns
                                        psl_ = pt[:, j, 0:c.BS]
                                        # split mask load DVE / Pool
                                        eng = (nc.vector if nmask % 2 == 0
                                               else nc.gpsimd)
                                        eng.tensor_mul(psl_, psl_, mw[:])
                                        nmask += 1
                                    pt_of[kc] = (pt, j, dtag)
                        for half, sl in enumerate(g):
                            qsl = slice(sl * c.BS, (sl + 1) * c.BS)
                            cl = slot_chunks[sl]
                            cpairs = pairup(cl)
                            po = pso.tile([65, c.BS], F32, tag="o")
                            for n, (kc0, kc1) in enumerate(cpairs):
                                pt, j0, dtag = pt_of[kc0]
                                assert pt_of[kc1][0] is pt and j0 == 0
                                col = 0 if dtag else half * c.BS
                                nc.tensor.matmul(
                                    po[:],
                                    vt[:, kc0:kc0 + 2,
                                       hp * 72:hp * 72 + 65],
                                    pt[:, :, col:col + c.BS],
                                    start=(n == 0),
                                    stop=(n == len(cpairs) - 1),
                                    perf_mode=DR)
                            # normalize by softmax denominator (row 64):
                            # reciprocal -> Pool partition-broadcast -> mul
                            rec = recp.tile([1, c.BS], F32, tag="rec")
                            with nc.allow_low_precision(
                                    reason="softmax denom reciprocal"):
                                nc.vector.reciprocal(rec[:], po[64:65, :])
                            bcr = recp.tile([64, c.BS], F32, tag="bcr")
                            nc.gpsimd.partition_broadcast(bcr[:], rec[:])
                            nc.vector.tensor_mul(
                                at[jj][base:base + 64, qsl], po[0:64, :],
                                bcr[:])

        es_qt.close()

        # ================= phase C: c_proj + residual =================
        es_x2 = ExitStack()
        x2pool = es_x2.enter_context(tc.tile_pool(name="x2pool", bufs=1, side="right"))
        x2t = []
        with (
            tc.tile_pool(name="cconst", bufs=1) as cconst,
            tc.tile_pool(name="ps_c", bufs=4, space="PSUM") as psc,
        ):
            bcp_r = cconst.tile([1, c.D], BF16)
            nc.sync.dma_start(out=bcp_r[:], in_=bcp_in[:, :])
            for tb in range(c.TB):
                x2 = x2pool.tile([128, c.D], F32, tag=f"x2_{tb}",
                                 name=f"x2_{tb}")
                for fh in range(c.FH):
                    fsl = slice(fh * c.VF, (fh + 1) * c.VF)
                    ps = psc.tile([128, c.VF], F32, tag="ps")
                    for i in range(c.DC):
                        nc.tensor.matmul(
                            ps[:], at[i][:, tb * 128:(tb + 1) * 128],
                            wc_all[:, i, fh * c.VF:(fh + 1) * c.VF],
                            start=(i == 0), stop=False)
                    # rank-1 bias add: ones^T @ b_cproj_eff
                    nc.tensor.matmul(ps[:], ones1_bf[:], bcp_r[0:1, fsl],
                                     start=False, stop=True)
                    nc.vector.tensor_add(x2[:, fsl], ps[:], xt[tb][:, fsl])
                x2t.append(x2)

        es_at.close()
        es_wc.close()
        es_x.close()

        # ================= phase D: LN2 + MLP =================
        with (
            tc.tile_pool(name="dconst", bufs=1) as dconst,
            tc.tile_pool(name="gt", bufs=1) as gtp,
            tc.tile_pool(name="wm", bufs=1) as wmp,
        ):
            ln2wc = dconst.tile([128, c.DC], F32)
            ln2bc = dconst.tile([128, c.DC], F32)
            for t, d in [(ln2wc, ln2wc_in), (ln2bc, ln2bc_in)]:
                nc.sync.dma_start(out=t[:], in_=d[:, :])
            bmp_r = dconst.tile([1, c.D], BF16)
            nc.sync.dma_start(out=bmp_r[:], in_=bmp_in[:, :])
            bfc = dconst.tile([128, c.GB], F32)
            nc.sync.dma_start(out=bfc[:], in_=bfc_in[:, :])

            # prefetch mproj weights early (scalar DMA queue, overlaps
            # cproj / LN2 / fc)
            wm_all, dwm_all = [], []
            for fh in range(c.FH):
                fsl = slice(fh * c.VF, (fh + 1) * c.VF)
                wm = wmp.tile([128, c.GB, c.VF], F8, tag=f"wm{fh}",
                              name=f"wm{fh}")
                nc.scalar.dma_start(
                    out=wm[:],
                    in_=w_mp8[:, fsl].rearrange(
                        "(g p) f -> p g f", p=128))
                wm_all.append(wm)
                dwm = wmp.tile([128, c.GB, c.VF], F8, tag=f"dwm{fh}",
                               name=f"dwm{fh}")
                nc.scalar.dma_start(
                    out=dwm[:],
                    in_=dw_mp8[:, fsl].rearrange(
                        "(g p) f -> p g f", p=128))
                dwm_all.append(dwm)

            GH = c.GB // 2  # mproj contraction halves
            NP = c.DC // 2

            def mm3(ps, wt, dwt, ht8, dht8, nw, wsl, hsl):
                """3-pass compensated-fp8 DoubleRow accumulation into ps:
                W8@h8 + W8@dh8 + dW8@h8 (all in the WS-scaled domain)."""
                passes = [(wt, ht8), (wt, dht8), (dwt, ht8)]
                for p, (wa, ha) in enumerate(passes):
                    for i in range(nw):
                        nc.tensor.matmul(
                            ps, wa[:, 2 * i:2 * i + 2, wsl],
                            ha[:, 2 * i:2 * i + 2, hsl],
                            start=(p == 0 and i == 0),
                            stop=(p == 2 and i == nw - 1),
                            perf_mode=DR)

            with (
                tc.tile_pool(name="mtp", bufs=1) as mtpool,
                tc.tile_pool(name="wf", bufs=2) as wfp,
            ):
                mt8 = mtpool.tile([128, c.DC, c.T], F8, name="mt8_all")
                dmt8 = mtpool.tile([128, c.DC, c.T], F8, name="dmt8_all")
                layernorm_t(x2t, ln2wc, ln2bc, mt8, "mt", dout_tile=dmt8)

                g8 = gtp.tile([128, c.GB, c.T], F8, name="g8_all")
                dg8 = gtp.tile([128, c.GB, c.T], F8, name="dg8_all")

                # ---------------- fc + gelu + mproj (lo half) ----------
                with (
                    tc.tile_pool(name="gtmp", bufs=4) as gtmpp,
                    tc.tile_pool(name="ps_g", bufs=4, space="PSUM") as psg,
                    tc.tile_pool(name="ps_m", bufs=4, space="PSUM") as psml,
                ):
                    GPW = 512 // 128  # g-blocks per W_fc slab
                    wf8_s = dwf8_s = None
                    for gb in range(c.GB):
                        if gb % GPW == 0:
                            j = gb // GPW
                            jsl = slice(j * 512, (j + 1) * 512)
                            wf8_s = wfp.tile([128, c.DC, 512], F8,
                                             tag="wf8", name=f"wf8_{gb}")
                            nc.scalar.dma_start(
                                out=wf8_s[:],
                                in_=w_fc8[:, jsl].rearrange(
                                    "(i p) f -> p i f", p=128))
                            dwf8_s = wfp.tile([128, c.DC, 512], F8,
                                              tag="dwf8", name=f"dwf8_{gb}")
                            nc.scalar.dma_start(
                                out=dwf8_s[:],
                                in_=dw_fc8[:, jsl].rearrange(
                                    "(i p) f -> p i f", p=128))
                        gl = (gb % GPW) * 128
                        for th in range(c.QH):
                            tsl = slice(th * c.QF, (th + 1) * c.QF)
                            ps = psg.tile([128, c.QF], F32, tag="ps")
                            mm3(ps[:], wf8_s, dwf8_s, mt8, dmt8, NP,
                                slice(gl, gl + 128), tsl)
                            gtmp = gtmpp.tile([128, c.QF], BF16, tag="gt")
                            nc.scalar.activation(
                                gtmp[:], ps[:], AF.Gelu_apprx_tanh,
                                bias=bfc[:, gb:gb + 1], scale=1.0 / WS)
                            nc.vector.tensor_copy(g8[:, gb, tsl], gtmp[:])
                            nc.vector.tensor_tensor(
                                out=dg8[:, gb, tsl], in0=gtmp[:],
                                in1=g8[:, gb, tsl], op=ALU.subtract)
                        if gb == GH - 1:
                            # low half of the mproj contraction starts while
                            # fc computes the high half; partial sums are
                            # accumulated into x2t (free after LN2)
                            for tb in range(c.TB):
                                tbs = slice(tb * 128, (tb + 1) * 128)
                                for fh in range(c.FH):
                                    fsl = slice(fh * c.VF, (fh + 1) * c.VF)
                                    ps = psml.tile([128, c.VF], F32,
                                                   tag="ps")
                                    mm3(ps[:], g8, dg8,
                                        wm_all[fh], dwm_all[fh], GH // 2,
                                        tbs, slice(0, c.VF))
                                    # descale on ACT (idle in the fc
                                    # window; DVE is the co-bottleneck)
                                    mtmp = gtmpp.tile([128, c.VF], F32,
                                                      tag="mtmp")
                                    nc.scalar.activation(
                                        mtmp[:], ps[:], AF.Identity,
                                        scale=1.0 / WS)
                                    nc.vector.tensor_add(
                                        x2t[tb][:, fsl], x2t[tb][:, fsl],
                                        mtmp[:])

            # ---------------- mproj (hi half) + residual ----------------
            with (
                tc.tile_pool(name="yout", bufs=3) as yop,
                tc.tile_pool(name="ps_m2", bufs=4, space="PSUM") as psm,
            ):
                for tb in range(c.TB):
                    tbs = slice(tb * 128, (tb + 1) * 128)
                    yo = yop.tile([128, c.D], F32, tag="yo")
                    for fh in range(c.FH):
                        fsl = slice(fh * c.VF, (fh + 1) * c.VF)
                        ps = psm.tile([128, c.VF], F32, tag="ps")
                        for p, (ga, wa) in enumerate(
                                [(g8, wm_all[fh]), (dg8, wm_all[fh]),
                                 (g8, dwm_all[fh])]):
                            for i in range(GH // 2):
                                g = GH + 2 * i
                                nc.tensor.matmul(
                                    ps[:], ga[:, g:g + 2, tbs],
                                    wa[:, g:g + 2, :],
                                    start=(p == 0 and i == 0), stop=False,
                                    perf_mode=DR)
                        nc.tensor.matmul(ps[:], ones1_bf[:],
                                         bmp_r[0:1, fsl],
                                         start=False, stop=True)
                        yt = yop.tile([128, c.VF], F32, tag="yt")
                        nc.scalar.activation(yt[:], ps[:], AF.Identity,
                                             scale=1.0 / WS)
                        nc.vector.tensor_add(yo[:, fsl], yt[:],
                                             x2t[tb][:, fsl])
                        # stream each feature-half out as soon as it's
                        # ready to shorten the final drain tail
                        nc.sync.dma_start(
                            out=y_out[tb * 128:(tb + 1) * 128, fsl],
                            in_=yo[:, fsl])

        es_x2.close()

    nc.compile()
    return nc


def make_core_inputs(cfg: Cfg, x, ln1_w, ln1_b, W_attn, b_attn, W_cproj,
                     b_cproj, ln2_w, ln2_b, W_fc, b_fc, W_mproj, b_mproj):
    """Split full inputs into one in_map per core."""
    c = cfg
    f32 = np.float32
    bf16 = ml_dtypes.bfloat16
    fp8 = ml_dtypes.float8_e4m3

    def lncol(v):
        return np.ascontiguousarray(
            np.asarray(v, f32).reshape(c.DC, 128).T)

    def comp8(W):
        """W*WS split into fp8 main + fp8 residual."""
        Ws = np.ascontiguousarray(np.asarray(W, f32) * WS)
        W8 = Ws.astype(fp8)
        dW8 = (Ws - W8.astype(f32)).astype(fp8)
        return W8, dW8

    b_v = np.asarray(b_attn[2 * c.D:3 * c.D], f32)
    bcp_eff = np.asarray(b_cproj, f32) + b_v @ np.asarray(W_cproj, f32)
    wf8, dwf8 = comp8(W_fc)
    wm8, dwm8 = comp8(W_mproj)
    shared = {
        "w_attn": np.ascontiguousarray(
            np.asarray(W_attn, f32) * WS).astype(fp8),
        "w_cproj": np.ascontiguousarray(W_cproj).astype(bf16),
        "w_fc8": wf8,
        "dw_fc8": dwf8,
        "w_mp8": wm8,
        "dw_mp8": dwm8,
        "ln1wc": lncol(ln1_w),
        "ln1bc": lncol(ln1_b),
        "ln2wc": lncol(ln2_w),
        "ln2bc": lncol(ln2_b),
        "bcp": np.ascontiguousarray(bcp_eff.reshape(1, c.D)).astype(bf16),
        # bmp lives in the WS-scaled mproj PSUM domain
        "bmp": np.ascontiguousarray(
            (np.asarray(b_mproj, f32) * WS).reshape(1, c.D)).astype(bf16),
        "bfc": np.ascontiguousarray(
            np.asarray(b_fc, f32).reshape(c.GB, 128).T),
    }
    bqk = np.asarray(b_attn[:2 * c.D], f32).reshape(2 * c.DC, 128).T.copy()
    bqk[:, :c.DC] *= 1.0 / math.sqrt(c.HD)
    shared["battn_qk"] = np.ascontiguousarray(bqk)

    in_maps = []
    for core in range(c.n_cores):
        b, half = core // 2, core % 2
        rows = core_rows(c, half)
        m = dict(shared)
        m["x"] = np.ascontiguousarray(np.asarray(x, f32)[b][rows])
        m["qidx"] = rows.astype(f32).reshape(1, c.T)
        kofs = np.empty((128, c.KC), f32)
        for kc in range(c.KC):
            parity = kc // c.KCH
            loc = (kc % c.KCH) * 128 + np.arange(128)
            kofs[:, kc] = (2 * (loc // c.BS) + parity) * c.BS + loc % c.BS
        m["kofs"] = kofs
        in_maps.append(m)
    return in_maps


def core_rows(cfg, half):
    """absolute sequence rows owned by a core with parity half"""
    c = cfg
    loc = np.arange(c.T)
    return (2 * (loc // c.BS) + half) * c.BS + loc % c.BS


_NC_CACHE = {}


def get_nc(cfg: Cfg):
    key = (cfg.B, cfg.S, cfg.D, cfg.H, cfg.F)
    if key not in _NC_CACHE:
        _NC_CACHE[key] = build(cfg)
    return _NC_CACHE[key]


def kernel(**inputs) -> np.ndarray:
    from concourse.bass_utils import run_bass_kernel_spmd

    cfg = Cfg()
    nc = get_nc(cfg)
    in_maps = make_core_inputs(cfg, **inputs)
    res = run_bass_kernel_spmd(nc, in_maps, core_ids=list(range(cfg.n_cores)))
    B, S, D, T = cfg.B, cfg.S, cfg.D, cfg.T
    out = np.empty((B, S, D), np.float32)
    for core in range(cfg.n_cores):
        b, half = core // 2, core % 2
        out[b, core_rows(cfg, half), :] = res.results[core]["y"]
    return out


# revision 74
# speedup vs baseline: 1.0012x; 1.0012x over previous
"""Single transformer block on 8 NeuronCores.

Sharding: core c handles batch b=c//2, sequence half c%2 (T=1024 tokens,
interleaved in stripes of BS=256 for causal load balance). All token-wise ops
(LN, QKV, c_proj, MLP) are purely local; attention needs the full sequence of
K/V per batch, obtained with a pairwise fp8 AllGather between cores
{2b, 2b+1}.

Dtype strategy (validated numerically, rel_l2 targets << 2e-2):
  - K, V, LN1 output h^T, W_attn: fp8e4m3 (W_attn host-prescaled by 50x so
    weights sit in the fp8 normal range; descaled at PSUM eviction)
  - QKV matmuls run fp8 DoubleRow (two 128-deep contraction subtiles per
    instruction)
  - everything else (q, P=exp(S), attention out, c_proj / fc / mproj weights
    and activations): bf16 inputs, f32 accumulation
  - residual stream x, x2: f32

Layout strategy (per core):
  - residual stream x: token-major [128t x D] SBUF tiles
  - h^T, m^T: feature-major via PE transpose; LN's affine (w, b) is folded
    into the per-partition scale/bias of the transpose eviction
  - scores computed transposed S^T[k, q]; softmax denominator comes free
    from an appended ones-column in V during the AV matmul; normalization
    uses a Pool partition_broadcast of the reciprocal row
  - causal mask applied as a 0/1 multiply on P=exp(S) boundary chunks only
  - exp is evaluated on chunk PAIRS ([128, 2, w] PSUM tiles) to halve the
    fixed per-call activation cost
  - biases: qkv biases via ACT eviction bias; b_v folded into an effective
    c_proj bias host-side (softmax rows sum to 1); c_proj/mproj biases added
    as rank-1 matmuls into PSUM; fc bias via gelu eviction bias
"""

import math
from contextlib import ExitStack

import numpy as np
import ml_dtypes

import concourse.bacc as bacc
import concourse.bass as bass
import concourse.mybir as mybir
import concourse.tile as tile
from concourse.masks import make_identity

F32 = mybir.dt.float32
F32R = mybir.dt.float32r
BF16 = mybir.dt.bfloat16
F8 = mybir.dt.float8e4
AF = mybir.ActivationFunctionType
ALU = mybir.AluOpType
DR = mybir.MatmulPerfMode.DoubleRow

EPS = 1e-5
WS = 50.0  # host-side W_attn scale so fp8 weights stay in normal range


class Cfg:
    def __init__(self, B=4, S=2048, D=1024, H=16, F=4096, n_cores=8, bs=256):
        self.B, self.S, self.D, self.H, self.F = B, S, D, H, F
        self.n_cores = n_cores
        assert n_cores == 2 * B
        self.HD = D // H
        assert self.HD == 64
        self.T = S // 2            # tokens per core
        self.TB = self.T // 128    # token 128-blocks
        self.DC = D // 128         # contraction chunks over D
        self.QF = min(512, self.T)  # q free-dim tile
        self.QH = self.T // self.QF
        self.KC = S // 128         # key 128-chunks over full sequence
        self.VF = min(512, D)      # out-feature tile for token-major outs
        self.FH = D // self.VF
        self.GB = F // 128         # MLP hidden 128-blocks
        self.HPB = 128 // self.HD  # heads per 128-feature block (=2)
        self.BS = min(bs, self.T)  # stripe block (q-slot) size
        self.SLOTS = self.T // self.BS
        self.KCH = self.KC // 2    # AG chunks per parity block
        self.CPB = self.BS // 128  # 128-chunks per stripe block


def chunk_absblk(c, kc):
    # absolute stripe-block index covered by AG chunk kc
    parity = kc // c.KCH
    loc = kc % c.KCH
    return 2 * ((loc * 128) // c.BS) + parity


def pairup(lst):
    """[(a,b), (c,d), ...] consecutive pairs; assumes even length."""
    assert len(lst) % 2 == 0
    return [(lst[i], lst[i + 1]) for i in range(0, len(lst), 2)]


def build(cfg: Cfg):
    c = cfg
    nc = bacc.Bacc(None, target_bir_lowering=False)

    # ---------------- I/O ----------------
    x_in = nc.dram_tensor("x", [c.T, c.D], F32, kind="ExternalInput")
    w_attn = nc.dram_tensor("w_attn", [c.D, 3 * c.D], F8, kind="ExternalInput")
    w_cproj = nc.dram_tensor("w_cproj", [c.D, c.D], BF16, kind="ExternalInput")
    # fc / mproj weights as compensated fp8 pairs (W*WS = W8 + dW8): the
    # GEMMs run 3 DoubleRow passes (W8@h8 + W8@dh8 + dW8@h8) at 0.75x the
    # bf16 PE cost and better-than-bf16 accuracy.
    w_fc8 = nc.dram_tensor("w_fc8", [c.D, c.F], F8, kind="ExternalInput")
    dw_fc8 = nc.dram_tensor("dw_fc8", [c.D, c.F], F8, kind="ExternalInput")
    w_mp8 = nc.dram_tensor("w_mp8", [c.F, c.D], F8, kind="ExternalInput")
    dw_mp8 = nc.dram_tensor("dw_mp8", [c.F, c.D], F8, kind="ExternalInput")
    ln1wc_in = nc.dram_tensor("ln1wc", [128, c.DC], F32, kind="ExternalInput")
    ln1bc_in = nc.dram_tensor("ln1bc", [128, c.DC], F32, kind="ExternalInput")
    ln2wc_in = nc.dram_tensor("ln2wc", [128, c.DC], F32, kind="ExternalInput")
    ln2bc_in = nc.dram_tensor("ln2bc", [128, c.DC], F32, kind="ExternalInput")
    battn_qk_in = nc.dram_tensor("battn_qk", [128, 2 * c.DC], F32,
                                 kind="ExternalInput")
    bcp_in = nc.dram_tensor("bcp", [1, c.D], BF16, kind="ExternalInput")
    bmp_in = nc.dram_tensor("bmp", [1, c.D], BF16, kind="ExternalInput")
    bfc_in = nc.dram_tensor("bfc", [128, c.GB], F32, kind="ExternalInput")
    qidx_in = nc.dram_tensor("qidx", [1, c.T], F32, kind="ExternalInput")
    kofs_in = nc.dram_tensor("kofs", [128, c.KC], F32, kind="ExternalInput")
    y_out = nc.dram_tensor("y", [c.T, c.D], F32, kind="ExternalOutput")

    pairs = [[2 * b, 2 * b + 1] for b in range(c.B)]

    def bcast(dram, p=128):
        # partition-broadcast DMA source: read row 0 for every partition
        return bass.AP(tensor=dram, offset=0, ap=[[0, p], [1, dram.shape[1]]])

    with tile.TileContext(nc) as tc, ExitStack() as es:
        dpool = es.enter_context(tc.tile_pool(name="dram", bufs=1, space="DRAM"))
        gconst = es.enter_context(tc.tile_pool(name="gconst", bufs=1))

        # DRAM bounce buffers for the pairwise AllGathers (fp8), split into
        # head-halves so each AG launches as soon as its half is produced
        # and attention unblocks incrementally.
        HB = c.D // 2  # feature rows per head-half
        kb_loc = [dpool.tile([HB, c.T], F8, name=f"kb_loc{h}")
                  for h in range(2)]
        kb_full = [dpool.tile([2 * HB, c.T], F8, name=f"kb_full{h}")
                   for h in range(2)]
        vb_loc = [dpool.tile([c.T, HB], F8, name=f"vb_loc{h}")
                  for h in range(2)]
        vb_full = [dpool.tile([2 * c.T, HB], F8, name=f"vb_full{h}")
                   for h in range(2)]

        # ---------------- global constants ----------------
        ident_bf = gconst.tile([128, 128], BF16)
        make_identity(nc, ident_bf[:])
        eps_t = gconst.tile([128, 1], F32)
        nc.vector.memset(eps_t[:], EPS)
        ones1_bf = gconst.tile([1, 128], BF16)
        nc.vector.memset(ones1_bf[:], 1.0)

        def layernorm_t(src_tiles, wcol, bcol, out_tile, out_tag,
                        dout_tile=None):
            """token-major LN over free axis + transpose to feature-major.

            Writes DC slices of out_tile [128, DC, T]; the LN affine (w, b)
            is applied per-partition at the transpose eviction. If dout_tile
            is given, also writes the fp8 quantization residual
            (exact - out) for compensated-fp8 GEMMs."""
            with (
                tc.tile_pool(name=f"ln_{out_tag}", bufs=3) as lnp,
                tc.tile_pool(name=f"ln2_{out_tag}", bufs=4) as lnp2,
                tc.tile_pool(name=f"ps_tr_{out_tag}", bufs=4,
                             space="PSUM") as ps_tr,
            ):
                for tb in range(c.TB):
                    src = src_tiles[tb]
                    nsg = c.D // 512 if c.D % 512 == 0 else 1
                    sgw = c.D // nsg
                    st = lnp.tile([128, nsg, 6], F32, tag="st")
                    for sg in range(nsg):
                        nc.vector.bn_stats(
                            out=st[:, sg, :],
                            in_=src[:, sg * sgw:(sg + 1) * sgw])
                    mv = lnp.tile([128, 2], F32, tag="mv")
                    nc.vector.bn_aggr(out=mv[:], in_=st[:])
                    sd = lnp.tile([128, 1], F32, tag="sd")
                    nc.scalar.activation(sd[:], mv[:, 1:2], AF.Sqrt,
                                         bias=eps_t[:, 0:1])
                    rs = lnp.tile([128, 1], F32, tag="rs")
                    nc.vector.reciprocal(rs[:], sd[:])
                    ht_ = lnp.tile([128, c.D], BF16, tag="h")
                    # normalize split DVE/Pool, sized by their relative
                    # throughput, to shorten the per-tile critical chain
                    hD = 640
                    nc.vector.tensor_scalar(
                        out=ht_[:, 0:hD], in0=src[:, 0:hD],
                        scalar1=mv[:, 0:1], scalar2=rs[:, 0:1],
                        op0=ALU.subtract, op1=ALU.mult)
                    nc.gpsimd.tensor_scalar(
                        out=ht_[:, hD:c.D], in0=src[:, hD:c.D],
                        scalar1=mv[:, 0:1], scalar2=rs[:, 0:1],
                        op0=ALU.subtract, op1=ALU.mult)
                    for i in range(c.DC):
                        pt = ps_tr.tile([128, 128], BF16, tag="tr")
                        nc.tensor.transpose(
                            pt[:], ht_[:, i * 128:(i + 1) * 128], ident_bf[:])
                        # ACT evict with the LN affine folded in as
                        # per-partition scale/bias (Pool cannot read PSUM)
                        tsl = slice(tb * 128, (tb + 1) * 128)
                        nc.scalar.activation(
                            out_tile[:, i, tsl], pt[:],
                            AF.Identity, bias=bcol[:, i:i + 1],
                            scale=wcol[:, i:i + 1])
                        if dout_tile is not None:
                            tmp = lnp2.tile([128, 128], BF16, tag="tmp")
                            nc.vector.tensor_scalar(
                                out=tmp[:], in0=pt[:],
                                scalar1=wcol[:, i:i + 1],
                                scalar2=bcol[:, i:i + 1],
                                op0=ALU.mult, op1=ALU.add)
                            nc.vector.tensor_tensor(
                                out=dout_tile[:, i, tsl], in0=tmp[:],
                                in1=out_tile[:, i, tsl],
                                op=ALU.subtract)

        # ================= phase A: LN1 + QKV =================
        es_x = ExitStack()
        xpool = es_x.enter_context(tc.tile_pool(name="xpool", bufs=1, side="left"))
        xt = []
        for tb in range(c.TB):
            t = xpool.tile([128, c.D], F32, tag=f"x{tb}", name=f"x{tb}")
            # alternate DMA queues and fetch halves so LN stats (which work
            # on 512-wide subgroups) start as early as possible
            eng = nc.sync if tb % 2 == 0 else nc.gpsimd
            for hx in range(2):
                csl = slice(hx * 512, (hx + 1) * 512)
                eng.dma_start(out=t[:, csl],
                              in_=x_in[tb * 128:(tb + 1) * 128, csl])
            xt.append(t)

        es_qt = ExitStack()
        qtpool = es_qt.enter_context(tc.tile_pool(name="qtpool", bufs=1, side="right"))
        qtp = []
        for j in range(c.H // c.HPB):
            qtp.append(qtpool.tile([128, c.T], BF16, tag=f"qt{j}",
                                   name=f"qt{j}"))

        with (
            tc.tile_pool(name="aconst", bufs=1) as aconst,
            tc.tile_pool(name="htp", bufs=1) as htpool,
        ):
            ln1wc = aconst.tile([128, c.DC], F32)
            ln1bc = aconst.tile([128, c.DC], F32)
            for t, d in [(ln1wc, ln1wc_in), (ln1bc, ln1bc_in)]:
                nc.sync.dma_start(out=t[:], in_=d[:, :])
            battn_qk = aconst.tile([128, 2 * c.DC], F32)
            nc.sync.dma_start(out=battn_qk[:], in_=battn_qk_in[:, :])

            NP = c.DC // 2  # DoubleRow contraction pairs

            with (
                tc.tile_pool(name="wa", bufs=3) as wap,
                tc.tile_pool(name="kout", bufs=3) as kop,
                tc.tile_pool(name="ps_mm", bufs=4, space="PSUM") as psmm,
            ):
                # allocate + fetch the QKV weights BEFORE the LN pools so
                # their SBUF space doesn't alias LN transients (which would
                # make the DMA wait for LN1 to release buffers)
                wk_all = wap.tile([128, c.DC, c.D], F8, tag="wa",
                                  name="wk_all")
                nc.sync.dma_start(
                    out=wk_all[:],
                    in_=w_attn[:, c.D:2 * c.D].rearrange(
                        "(i p) f -> p i f", p=128))
                wv_all = wap.tile([128, c.DC, c.D], F8, tag="wa",
                                  name="wv_all")
                nc.sync.dma_start(
                    out=wv_all[:],
                    in_=w_attn[:, 2 * c.D:3 * c.D].rearrange(
                        "(i p) f -> p i f", p=128))

                ht = htpool.tile([128, c.DC, c.T], F8, name="ht_all")
                layernorm_t(xt, ln1wc, ln1bc, ht, "ht")

                # ---- K and V passes, interleaved by head-half so the
                # collective order is K0, V0, K1, V1 (attention for the
                # first head-half unblocks while the second half transfers)
                MH = c.DC // 2  # m-chunks per head-half
                for kh in range(2):
                    # k^T pass for this head-half (feature-major)
                    for ml in range(MH):
                        m = kh * MH + ml
                        for th in range(c.QH):
                            ps = psmm.tile([128, c.QF], F32, tag="ps")
                            for i in range(NP):
                                nc.tensor.matmul(
                                    ps[:], wk_all[:, 2 * i:2 * i + 2,
                                                  m * 128:(m + 1) * 128],
                                    ht[:, 2 * i:2 * i + 2,
                                       th * c.QF:(th + 1) * c.QF],
                                    start=(i == 0), stop=(i == NP - 1),
                                    perf_mode=DR)
                            ko = kop.tile([128, c.QF], F8, tag="ko")
                            # alternate ACT/DVE so the eviction tail on the
                            # path to the K AllGather halves
                            if (m + th) % 2 == 0:
                                nc.scalar.activation(
                                    ko[:], ps[:], AF.Identity,
                                    bias=battn_qk[:, c.DC + m:c.DC + m + 1],
                                    scale=1.0 / WS)
                            else:
                                nc.vector.tensor_scalar(
                                    out=ko[:], in0=ps[:], scalar1=1.0 / WS,
                                    scalar2=battn_qk[:, c.DC + m:
                                                     c.DC + m + 1],
                                    op0=ALU.mult, op1=ALU.add)
                            nc.sync.dma_start(
                                out=kb_loc[kh][ml * 128:(ml + 1) * 128,
                                               th * c.QF:(th + 1) * c.QF],
                                in_=ko[:])
                    nc.gpsimd.collective_compute(
                        "AllGather", ALU.bypass, ins=[kb_loc[kh][:]],
                        outs=[kb_full[kh][:]], replica_groups=pairs)
                    # v pass for this head-half (token-major)
                    for tb in range(c.TB):
                        ps = psmm.tile([128, c.VF], F32, tag="ps")
                        for i in range(NP):
                            nc.tensor.matmul(
                                ps[:], ht[:, 2 * i:2 * i + 2,
                                          tb * 128:(tb + 1) * 128],
                                wv_all[:, 2 * i:2 * i + 2,
                                       kh * c.VF:(kh + 1) * c.VF],
                                start=(i == 0), stop=(i == NP - 1),
                                perf_mode=DR)
                        vo = kop.tile([128, c.VF], F8, tag="vo")
                        nc.vector.tensor_scalar(
                            out=vo[:], in0=ps[:], scalar1=1.0 / WS,
                            scalar2=None, op0=ALU.mult)
                        nc.sync.dma_start(
                            out=vb_loc[kh][tb * 128:(tb + 1) * 128, :],
                            in_=vo[:])
                    nc.gpsimd.collective_compute(
                        "AllGather", ALU.bypass, ins=[vb_loc[kh][:]],
                        outs=[vb_full[kh][:]], replica_groups=pairs)

                # ---- q^T pass (feature-major, stays in SBUF) ----
                wq_all = wap.tile([128, c.DC, c.D], F8, tag="wa",
                                  name="wq_all")
                nc.sync.dma_start(
                    out=wq_all[:],
                    in_=w_attn[:, 0:c.D].rearrange(
                        "(i p) f -> p i f", p=128))
                for m in range(c.DC):
                    for th in range(c.QH):
                        ps = psmm.tile([128, c.QF], F32, tag="ps")
                        for i in range(NP):
                            nc.tensor.matmul(
                                ps[:], wq_all[:, 2 * i:2 * i + 2,
                                              m * 128:(m + 1) * 128],
                                ht[:, 2 * i:2 * i + 2,
                                   th * c.QF:(th + 1) * c.QF],
                                start=(i == 0), stop=(i == NP - 1),
                                perf_mode=DR)
                        # scale by 1/sqrt(HD) at eviction (bias pre-scaled)
                        nc.scalar.activation(
                            qtp[m][:, th * c.QF:(th + 1) * c.QF], ps[:],
                            AF.Identity, bias=battn_qk[:, m:m + 1],
                            scale=1.0 / (WS * math.sqrt(c.HD)))

        # ================= phase B: attention =================
        # prefetch c_proj weights during attention (scalar DMA ring)
        es_wc = ExitStack()
        wcp = es_wc.enter_context(tc.tile_pool(name="wc", bufs=1, side="left"))
        wc_all = wcp.tile([128, c.DC, c.D], BF16, tag="wc", name="wc_all")
        nc.scalar.dma_start(
            out=wc_all[:],
            in_=w_cproj[:, :].rearrange("(i p) f -> p i f", p=128))

        es_at = ExitStack()
        atpool = es_at.enter_context(tc.tile_pool(name="atpool", bufs=1, side="left"))
        at = []
        for j in range(c.DC):
            at.append(atpool.tile([128, c.T], BF16, tag=f"at{j}",
                                  name=f"at{j}"))

        with (
            tc.tile_pool(name="bconst", bufs=1) as bconst,
            tc.tile_pool(name="mask", bufs=1) as maskp,
            tc.tile_pool(name="kv", bufs=5) as kvp,
            tc.tile_pool(name="pt5", bufs=56) as ptp5,
            tc.tile_pool(name="pt2", bufs=28) as ptp2,
            tc.tile_pool(name="rec", bufs=4) as recp,
            tc.tile_pool(name="ps_s", bufs=3, space="PSUM") as pss,
            tc.tile_pool(name="ps_o", bufs=2, space="PSUM") as pso,
        ):
            qidx = bconst.tile([128, c.T], F32)
            nc.sync.dma_start(out=qidx[:], in_=bcast(qidx_in))
            kofs = bconst.tile([128, c.KC], F32)
            nc.sync.dma_start(out=kofs[:], in_=kofs_in[:, :])

            # per-slot chunk lists (compile-time causal structure)
            slot_chunks = []
            for sl in range(c.SLOTS):
                cl = [kc for kc in range(c.KC)
                      if chunk_absblk(c, kc) <= 2 * sl + 1]
                slot_chunks.append(cl)

            # group q-slots in pairs: one 512-wide QK/exp per k-chunk
            groups = []
            sl = 0
            while sl < c.SLOTS:
                g = [sl, sl + 1] if sl + 1 < c.SLOTS else [sl]
                groups.append(g)
                sl += len(g)

            # pre-generate boundary masks per (group, chunk) where the chunk
            # may cross the causal diagonal. For "full"-class boundary chunks
            # (absblk <= 2*g0+1) only the LOWER slot of the group can be
            # non-visible (the upper slot's stripes sit strictly after the
            # chunk), so every mask is one slot (BS) wide: full-class masks
            # cover slot g0's columns, diff-class masks cover slot g1's.
            masks = {}
            for gi, g in enumerate(groups):
                for kc in slot_chunks[g[-1]]:
                    ab = chunk_absblk(c, kc)
                    if ab < 2 * g[0]:
                        continue
                    msl_slot = g[0] if ab <= 2 * g[0] + 1 else g[-1]
                    qsl = slice(msl_slot * c.BS, (msl_slot + 1) * c.BS)
                    mk = maskp.tile([128, c.BS], BF16,
                                    tag=f"mk{gi}_{kc}",
                                    name=f"mk{gi}_{kc}")
                    nc.vector.tensor_scalar(
                        out=mk[:], in0=qidx[:, qsl],
                        scalar1=kofs[:, kc:kc + 1], scalar2=None,
                        op0=ALU.is_ge)
                    masks[(gi, kc)] = mk

            for jj in range(c.H // c.HPB):
                kh = jj // 4        # head-half buffer index
                jl = jj % 4
                ktp = kvp.tile([128, c.S], F8, tag="ktp")
                for hp in range(c.HPB):
                    hl = 2 * jl + hp  # head within the half
                    psl = slice(hp * 64, hp * 64 + 64)
                    nc.sync.dma_start(
                        out=ktp[psl, 0:c.T],
                        in_=kb_full[kh][64 * hl:64 * hl + 64, :])
                    nc.sync.dma_start(
                        out=ktp[psl, c.T:c.S],
                        in_=kb_full[kh][HB + 64 * hl:HB + 64 * hl + 64, :])
                # V for both heads of the pair, with an appended ones column
                # per head. Chunk row padded to 144 so the DoubleRow
                # Ldweights outer step is 16B-aligned (head slots at 0, 72).
                vt = kvp.tile([128, c.KC, 144], F8, tag="vt")
                vt4 = vt[:, :, :].rearrange("p kc (h f) -> p kc h f", f=72)
                for hp in range(c.HPB):
                    fb = 128 * jl + 64 * hp
                    nc.sync.dma_start(
                        out=vt[:, :, hp * 72:hp * 72 + 64],
                        in_=vb_full[kh][:, fb:fb + 64].rearrange(
                            "(kc p) f -> p kc f", p=128))
                nc.gpsimd.memset(vt4[:, :, :, 64:65], 1.0)

                for hp in range(c.HPB):
                    base = hp * 64
                    for gi, g in enumerate(groups):
                        gw = len(g) * c.BS
                        gq = slice(g[0] * c.BS, g[0] * c.BS + gw)
                        rhs_q = qtp[jj][base:base + 64, gq]
                        cl_all = slot_chunks[g[-1]]
                        full = [kc for kc in cl_all
                                if not (len(g) == 2 and
                                        chunk_absblk(c, kc) > 2 * g[0] + 1)]
                        diff = [kc for kc in cl_all if kc not in full]
                        pt_of = {}
                        nmask = 0
                        for plist, w, dtag in ((full, gw, False),
                                               (diff, c.BS, True)):
                            rq = (qtp[jj][base:base + 64,
                                          g[1] * c.BS:(g[1] + 1) * c.BS]
                                  if dtag else rhs_q)
                            for kc0, kc1 in pairup(plist):
                                ps = pss.tile([128, 2, gw], F32, tag="s")
                                for j, kc in ((0, kc0), (1, kc1)):
                                    nc.tensor.matmul(
                                        ps[:, j, 0:w],
                                        ktp[base:base + 64,
                                            kc * 128:(kc + 1) * 128],
                                        rq, start=True, stop=True)
                                ptpool = ptp5 if w == gw else ptp2
                                pt = ptpool.tile([128, 2, w], F8,
                                                 tag=f"pt{w}")
                                nc.scalar.activation(pt[:, :, :],
                                                     ps[:, :, 0:w], AF.Exp)
                                for j, kc in ((0, kc0), (1, kc1)):
                                    if (gi, kc) in masks:
                                        mw = masks[(gi, kc)]
                                        # full-class masks only touch the
                                        # lower slot's BS columns
                                        psl_ = pt[:, j, 0:c.BS]
                                        # split mask load DVE / Pool
                                        eng = (nc.vector if nmask % 2 == 0
                                               else nc.gpsimd)
                                        eng.tensor_mul(psl_, psl_, mw[:])
                                        nmask += 1
                                    pt_of[kc] = (pt, j, dtag)
                        for half, sl in enumerate(g):
                            qsl = slice(sl * c.BS, (sl + 1) * c.BS)
                            cl = slot_chunks[sl]
                            cpairs = pairup(cl)
                            po = pso.tile([65, c.BS], F32, tag="o")
                            for n, (kc0, kc1) in enumerate(cpairs):
                                pt, j0, dtag = pt_of[kc0]
                                assert pt_of[kc1][0] is pt and j0 == 0
                                col = 0 if dtag else half * c.BS
                                nc.tensor.matmul(
                                    po[:],
                                    vt[:, kc0:kc0 + 2,
                                       hp * 72:hp * 72 + 65],
                                    pt[:, :, col:col + c.BS],
                                    start=(n == 0),
                                    stop=(n == len(cpairs) - 1),
                                    perf_mode=DR)
                            # normalize by softmax denominator (row 64):
                            # reciprocal -> Pool partition-broadcast -> mul
                            rec = recp.tile([1, c.BS], F32, tag="rec")
                            with nc.allow_low_precision(
                                    reason="softmax denom reciprocal"):
                                nc.vector.reciprocal(rec[:], po[64:65, :])
                            bcr = recp.tile([64, c.BS], F32, tag="bcr")
                            nc.gpsimd.partition_broadcast(bcr[:], rec[:])
                            nc.vector.tensor_mul(
                                at[jj][base:base + 64, qsl], po[0:64, :],
                                bcr[:])

        es_qt.close()

        # ================= phase C: c_proj + residual =================
        es_x2 = ExitStack()
        x2pool = es_x2.enter_context(tc.tile_pool(name="x2pool", bufs=1, side="right"))
        x2t = []
        with (
            tc.tile_pool(name="cconst", bufs=1) as cconst,
            tc.tile_pool(name="ps_c", bufs=4, space="PSUM") as psc,
        ):
            bcp_r = cconst.tile([1, c.D], BF16)
            nc.sync.dma_start(out=bcp_r[:], in_=bcp_in[:, :])
            for tb in range(c.TB):
                x2 = x2pool.tile([128, c.D], F32, tag=f"x2_{tb}",
                                 name=f"x2_{tb}")
                for fh in range(c.FH):
                    fsl = slice(fh * c.VF, (fh + 1) * c.VF)
                    ps = psc.tile([128, c.VF], F32, tag="ps")
                    for i in range(c.DC):
                        nc.tensor.matmul(
                            ps[:], at[i][:, tb * 128:(tb + 1) * 128],
                            wc_all[:, i, fh * c.VF:(fh + 1) * c.VF],
                            start=(i == 0), stop=False)
                    # rank-1 bias add: ones^T @ b_cproj_eff
                    nc.tensor.matmul(ps[:], ones1_bf[:], bcp_r[0:1, fsl],
                                     start=False, stop=True)
                    nc.vector.tensor_add(x2[:, fsl], ps[:], xt[tb][:, fsl])
                x2t.append(x2)

        es_at.close()
        es_wc.close()
        es_x.close()

        # ================= phase D: LN2 + MLP =================
        with (
            tc.tile_pool(name="dconst", bufs=1) as dconst,
            tc.tile_pool(name="gt", bufs=1) as gtp,
            tc.tile_pool(name="wm", bufs=1) as wmp,
        ):
            ln2wc = dconst.tile([128, c.DC], F32)
            ln2bc = dconst.tile([128, c.DC], F32)
            for t, d in [(ln2wc, ln2wc_in), (ln2bc, ln2bc_in)]:
                nc.sync.dma_start(out=t[:], in_=d[:, :])
            bmp_r = dconst.tile([1, c.D], BF16)
            nc.sync.dma_start(out=bmp_r[:], in_=bmp_in[:, :])
            bfc = dconst.tile([128, c.GB], F32)
            nc.sync.dma_start(out=bfc[:], in_=bfc_in[:, :])

            # prefetch mproj weights early (scalar DMA queue, overlaps
            # cproj / LN2 / fc)
            wm_all, dwm_all = [], []
            for fh in range(c.FH):
                fsl = slice(fh * c.VF, (fh + 1) * c.VF)
                wm = wmp.tile([128, c.GB, c.VF], F8, tag=f"wm{fh}",
                              name=f"wm{fh}")
                nc.scalar.dma_start(
                    out=wm[:],
                    in_=w_mp8[:, fsl].rearrange(
                        "(g p) f -> p g f", p=128))
                wm_all.append(wm)
                dwm = wmp.tile([128, c.GB, c.VF], F8, tag=f"dwm{fh}",
                               name=f"dwm{fh}")
                nc.scalar.dma_start(
                    out=dwm[:],
                    in_=dw_mp8[:, fsl].rearrange(
                        "(g p) f -> p g f", p=128))
                dwm_all.append(dwm)

            GH = c.GB // 2  # mproj contraction halves
            NP = c.DC // 2

            def mm3(ps, wt, dwt, ht8, dht8, nw, wsl, hsl):
                """3-pass compensated-fp8 DoubleRow accumulation into ps:
                W8@h8 + W8@dh8 + dW8@h8 (all in the WS-scaled domain)."""
                passes = [(wt, ht8), (wt, dht8), (dwt, ht8)]
                for p, (wa, ha) in enumerate(passes):
                    for i in range(nw):
                        nc.tensor.matmul(
                            ps, wa[:, 2 * i:2 * i + 2, wsl],
                            ha[:, 2 * i:2 * i + 2, hsl],
                            start=(p == 0 and i == 0),
                            stop=(p == 2 and i == nw - 1),
                            perf_mode=DR)

            with (
                tc.tile_pool(name="mtp", bufs=1) as mtpool,
                tc.tile_pool(name="wf", bufs=2) as wfp,
            ):
                mt8 = mtpool.tile([128, c.DC, c.T], F8, name="mt8_all")
                dmt8 = mtpool.tile([128, c.DC, c.T], F8, name="dmt8_all")
                layernorm_t(x2t, ln2wc, ln2bc, mt8, "mt", dout_tile=dmt8)

                g8 = gtp.tile([128, c.GB, c.T], F8, name="g8_all")
                dg8 = gtp.tile([128, c.GB, c.T], F8, name="dg8_all")

                # ---------------- fc + gelu + mproj (lo half) ----------
                with (
                    tc.tile_pool(name="gtmp", bufs=4) as gtmpp,
                    tc.tile_pool(name="ps_g", bufs=4, space="PSUM") as psg,
                    tc.tile_pool(name="ps_m", bufs=4, space="PSUM") as psml,
                ):
                    GPW = 512 // 128  # g-blocks per W_fc slab
                    wf8_s = dwf8_s = None
                    for gb in range(c.GB):
                        if gb % GPW == 0:
                            j = gb // GPW
                            jsl = slice(j * 512, (j + 1) * 512)
                            wf8_s = wfp.tile([128, c.DC, 512], F8,
                                             tag="wf8", name=f"wf8_{gb}")
                            nc.scalar.dma_start(
                                out=wf8_s[:],
                                in_=w_fc8[:, jsl].rearrange(
                                    "(i p) f -> p i f", p=128))
                            dwf8_s = wfp.tile([128, c.DC, 512], F8,
                                              tag="dwf8", name=f"dwf8_{gb}")
                            nc.scalar.dma_start(
                                out=dwf8_s[:],
                                in_=dw_fc8[:, jsl].rearrange(
                                    "(i p) f -> p i f", p=128))
                        gl = (gb % GPW) * 128
                        for th in range(c.QH):
                            tsl = slice(th * c.QF, (th + 1) * c.QF)
                            ps = psg.tile([128, c.QF], F32, tag="ps")
                            mm3(ps[:], wf8_s, dwf8_s, mt8, dmt8, NP,
                                slice(gl, gl + 128), tsl)
                            gtmp = gtmpp.tile([128, c.QF], BF16, tag="gt")
                            nc.scalar.activation(
                                gtmp[:], ps[:], AF.Gelu_apprx_tanh,
                                bias=bfc[:, gb:gb + 1], scale=1.0 / WS)
                            nc.vector.tensor_copy(g8[:, gb, tsl], gtmp[:])
                            nc.vector.tensor_tensor(
                                out=dg8[:, gb, tsl], in0=gtmp[:],
                                in1=g8[:, gb, tsl], op=ALU.subtract)
                        if gb == GH - 1:
                            # low half of the mproj contraction starts while
                            # fc computes the high half; partial sums are
                            # accumulated into x2t (free after LN2)
                            for tb in range(c.TB):
                                tbs = slice(tb * 128, (tb + 1) * 128)
                                for fh in range(c.FH):
                                    fsl = slice(fh * c.VF, (fh + 1) * c.VF)
                                    ps = psml.tile([128, c.VF], F32,
                                                   tag="ps")
                                    mm3(ps[:], g8, dg8,
                                        wm_all[fh], dwm_all[fh], GH // 2,
                                        tbs, slice(0, c.VF))
                                    mtmp = gtmpp.tile([128, c.VF], F32,
                                                      tag="mtmp")
                                    nc.vector.tensor_scalar(
                                        out=mtmp[:], in0=ps[:],
                                        scalar1=1.0 / WS, scalar2=None,
                                        op0=ALU.mult)
                                    nc.vector.tensor_add(
                                        x2t[tb][:, fsl], x2t[tb][:, fsl],
                                        mtmp[:])

            # ---------------- mproj (hi half) + residual ----------------
            with (
                tc.tile_pool(name="yout", bufs=3) as yop,
                tc.tile_pool(name="ps_m2", bufs=4, space="PSUM") as psm,
            ):
                for tb in range(c.TB):
                    tbs = slice(tb * 128, (tb + 1) * 128)
                    yo = yop.tile([128, c.D], F32, tag="yo")
                    for fh in range(c.FH):
                        fsl = slice(fh * c.VF, (fh + 1) * c.VF)
                        ps = psm.tile([128, c.VF], F32, tag="ps")
                        for p, (ga, wa) in enumerate(
                                [(g8, wm_all[fh]), (dg8, wm_all[fh]),
                                 (g8, dwm_all[fh])]):
                            for i in range(GH // 2):
                                g = GH + 2 * i
                                nc.tensor.matmul(
                                    ps[:], ga[:, g:g + 2, tbs],
                                    wa[:, g:g + 2, :],
                                    start=(p == 0 and i == 0), stop=False,
                                    perf_mode=DR)
                        nc.tensor.matmul(ps[:], ones1_bf[:],
                                         bmp_r[0:1, fsl],
                                         start=False, stop=True)
                        yt = yop.tile([128, c.VF], F32, tag="yt")
                        nc.vector.tensor_scalar(
                            out=yt[:], in0=ps[:], scalar1=1.0 / WS,
                            scalar2=None, op0=ALU.mult)
                        nc.vector.tensor_add(yo[:, fsl], yt[:],
                                             x2t[tb][:, fsl])
                        # stream each feature-half out as soon as it's
                        # ready to shorten the final drain tail
                        nc.sync.dma_start(
                            out=y_out[tb * 128:(tb + 1) * 128, fsl],
                            in_=yo[:, fsl])

        es_x2.close()

    nc.compile()
    return nc


def make_core_inputs(cfg: Cfg, x, ln1_w, ln1_b, W_attn, b_attn, W_cproj,
                     b_cproj, ln2_w, ln2_b, W_fc, b_fc, W_mproj, b_mproj):
    """Split full inputs into one in_map per core."""
    c = cfg
    f32 = np.float32
    bf16 = ml_dtypes.bfloat16
    fp8 = ml_dtypes.float8_e4m3

    def lncol(v):
        return np.ascontiguousarray(
            np.asarray(v, f32).reshape(c.DC, 128).T)

    def comp8(W):
        """W*WS split into fp8 main + fp8 residual."""
        Ws = np.ascontiguousarray(np.asarray(W, f32) * WS)
        W8 = Ws.astype(fp8)
        dW8 = (Ws - W8.astype(f32)).astype(fp8)
        return W8, dW8

    b_v = np.asarray(b_attn[2 * c.D:3 * c.D], f32)
    bcp_eff = np.asarray(b_cproj, f32) + b_v @ np.asarray(W_cproj, f32)
    wf8, dwf8 = comp8(W_fc)
    wm8, dwm8 = comp8(W_mproj)
    shared = {
        "w_attn": np.ascontiguousarray(
            np.asarray(W_attn, f32) * WS).astype(fp8),
        "w_cproj": np.ascontiguousarray(W_cproj).astype(bf16),
        "w_fc8": wf8,
        "dw_fc8": dwf8,
        "w_mp8": wm8,
        "dw_mp8": dwm8,
        "ln1wc": lncol(ln1_w),
        "ln1bc": lncol(ln1_b),
        "ln2wc": lncol(ln2_w),
        "ln2bc": lncol(ln2_b),
        "bcp": np.ascontiguousarray(bcp_eff.reshape(1, c.D)).astype(bf16),
        # bmp lives in the WS-scaled mproj PSUM domain
        "bmp": np.ascontiguousarray(
            (np.asarray(b_mproj, f32) * WS).reshape(1, c.D)).astype(bf16),
        "bfc": np.ascontiguousarray(
            np.asarray(b_fc, f32).reshape(c.GB, 128).T),
    }
    bqk = np.asarray(b_attn[:2 * c.D], f32).reshape(2 * c.DC, 128).T.copy()
    bqk[:, :c.DC] *= 1.0 / math.sqrt(c.HD)
    shared["battn_qk"] = np.ascontiguousarray(bqk)

    in_maps = []
    for core in range(c.n_cores):
        b, half = core // 2, core % 2
        rows = core_rows(c, half)
        m = dict(shared)
        m["x"] = np.ascontiguousarray(np.asarray(x, f32)[b][rows])
        m["qidx"] = rows.astype(f32).reshape(1, c.T)
        kofs = np.empty((128, c.KC), f32)
        for kc in range(c.KC):
            parity = kc // c.KCH
            loc = (kc % c.KCH) * 128 + np.arange(128)
            kofs[:, kc] = (2 * (loc // c.BS) + parity) * c.BS + loc % c.BS
        m["kofs"] = kofs
        in_maps.append(m)
    return in_maps


def core_rows(cfg, half):
    """absolute sequence rows owned by a core with parity half"""
    c = cfg
    loc = np.arange(c.T)
    return (2 * (loc // c.BS) + half) * c.BS + loc % c.BS


_NC_CACHE = {}


def get_nc(cfg: Cfg):
    key = (cfg.B, cfg.S, cfg.D, cfg.H, cfg.F)
    if key not in _NC_CACHE:
        _NC_CACHE[key] = build(cfg)
    return _NC_CACHE[key]


def kernel(**inputs) -> np.ndarray:
    from concourse.bass_utils import run_bass_kernel_spmd

    cfg = Cfg()
    nc = get_nc(cfg)
    in_maps = make_core_inputs(cfg, **inputs)
    res = run_bass_kernel_spmd(nc, in_maps, core_ids=list(range(cfg.n_cores)))
    B, S, D, T = cfg.B, cfg.S, cfg.D, cfg.T
    out = np.empty((B, S, D), np.float32)
    for core in range(cfg.n_cores):
        b, half = core // 2, core % 2
        out[b, core_rows(cfg, half), :] = res.results[core]["y"]
    return out


# revision 75
# speedup vs baseline: 1.0059x; 1.0047x over previous
"""Single transformer block on 8 NeuronCores.

Sharding: core c handles batch b=c//2, sequence half c%2 (T=1024 tokens,
interleaved in stripes of BS=256 for causal load balance). All token-wise ops
(LN, QKV, c_proj, MLP) are purely local; attention needs the full sequence of
K/V per batch, obtained with a pairwise fp8 AllGather between cores
{2b, 2b+1}.

Dtype strategy (validated numerically, rel_l2 targets << 2e-2):
  - K, V, LN1 output h^T, W_attn: fp8e4m3 (W_attn host-prescaled by 50x so
    weights sit in the fp8 normal range; descaled at PSUM eviction)
  - QKV matmuls run fp8 DoubleRow (two 128-deep contraction subtiles per
    instruction)
  - everything else (q, P=exp(S), attention out, c_proj / fc / mproj weights
    and activations): bf16 inputs, f32 accumulation
  - residual stream x, x2: f32

Layout strategy (per core):
  - residual stream x: token-major [128t x D] SBUF tiles
  - h^T, m^T: feature-major via PE transpose; LN's affine (w, b) is folded
    into the per-partition scale/bias of the transpose eviction
  - scores computed transposed S^T[k, q]; softmax denominator comes free
    from an appended ones-column in V during the AV matmul; normalization
    uses a Pool partition_broadcast of the reciprocal row
  - causal mask applied as a 0/1 multiply on P=exp(S) boundary chunks only
  - exp is evaluated on chunk PAIRS ([128, 2, w] PSUM tiles) to halve the
    fixed per-call activation cost
  - biases: qkv biases via ACT eviction bias; b_v folded into an effective
    c_proj bias host-side (softmax rows sum to 1); c_proj/mproj biases added
    as rank-1 matmuls into PSUM; fc bias via gelu eviction bias
"""

import math
from contextlib import ExitStack

import numpy as np
import ml_dtypes

import concourse.bacc as bacc
import concourse.bass as bass
import concourse.mybir as mybir
import concourse.tile as tile
from concourse.masks import make_identity

F32 = mybir.dt.float32
F32R = mybir.dt.float32r
BF16 = mybir.dt.bfloat16
F8 = mybir.dt.float8e4
AF = mybir.ActivationFunctionType
ALU = mybir.AluOpType
DR = mybir.MatmulPerfMode.DoubleRow

EPS = 1e-5
WS = 50.0  # host-side W_attn scale so fp8 weights stay in normal range


class Cfg:
    def __init__(self, B=4, S=2048, D=1024, H=16, F=4096, n_cores=8, bs=256):
        self.B, self.S, self.D, self.H, self.F = B, S, D, H, F
        self.n_cores = n_cores
        assert n_cores == 2 * B
        self.HD = D // H
        assert self.HD == 64
        self.T = S // 2            # tokens per core
        self.TB = self.T // 128    # token 128-blocks
        self.DC = D // 128         # contraction chunks over D
        self.QF = min(512, self.T)  # q free-dim tile
        self.QH = self.T // self.QF
        self.KC = S // 128         # key 128-chunks over full sequence
        self.VF = min(512, D)      # out-feature tile for token-major outs
        self.FH = D // self.VF
        self.GB = F // 128         # MLP hidden 128-blocks
        self.HPB = 128 // self.HD  # heads per 128-feature block (=2)
        self.BS = min(bs, self.T)  # stripe block (q-slot) size
        self.SLOTS = self.T // self.BS
        self.KCH = self.KC // 2    # AG chunks per parity block
        self.CPB = self.BS // 128  # 128-chunks per stripe block


def chunk_absblk(c, kc):
    # absolute stripe-block index covered by AG chunk kc
    parity = kc // c.KCH
    loc = kc % c.KCH
    return 2 * ((loc * 128) // c.BS) + parity


def pairup(lst):
    """[(a,b), (c,d), ...] consecutive pairs; assumes even length."""
    assert len(lst) % 2 == 0
    return [(lst[i], lst[i + 1]) for i in range(0, len(lst), 2)]


def build(cfg: Cfg):
    c = cfg
    nc = bacc.Bacc(None, target_bir_lowering=False)

    # ---------------- I/O ----------------
    x_in = nc.dram_tensor("x", [c.T, c.D], F32, kind="ExternalInput")
    w_attn = nc.dram_tensor("w_attn", [c.D, 3 * c.D], F8, kind="ExternalInput")
    w_cproj = nc.dram_tensor("w_cproj", [c.D, c.D], BF16, kind="ExternalInput")
    # fc / mproj weights as compensated fp8 pairs (W*WS = W8 + dW8): the
    # GEMMs run 3 DoubleRow passes (W8@h8 + W8@dh8 + dW8@h8) at 0.75x the
    # bf16 PE cost and better-than-bf16 accuracy.
    w_fc8 = nc.dram_tensor("w_fc8", [c.D, c.F], F8, kind="ExternalInput")
    dw_fc8 = nc.dram_tensor("dw_fc8", [c.D, c.F], F8, kind="ExternalInput")
    w_mp8 = nc.dram_tensor("w_mp8", [c.F, c.D], F8, kind="ExternalInput")
    dw_mp8 = nc.dram_tensor("dw_mp8", [c.F, c.D], F8, kind="ExternalInput")
    ln1wc_in = nc.dram_tensor("ln1wc", [128, c.DC], F32, kind="ExternalInput")
    ln1bc_in = nc.dram_tensor("ln1bc", [128, c.DC], F32, kind="ExternalInput")
    ln2wc_in = nc.dram_tensor("ln2wc", [128, c.DC], F32, kind="ExternalInput")
    ln2bc_in = nc.dram_tensor("ln2bc", [128, c.DC], F32, kind="ExternalInput")
    battn_qk_in = nc.dram_tensor("battn_qk", [128, 2 * c.DC], F32,
                                 kind="ExternalInput")
    bcp_in = nc.dram_tensor("bcp", [1, c.D], BF16, kind="ExternalInput")
    bmp_in = nc.dram_tensor("bmp", [1, c.D], BF16, kind="ExternalInput")
    bfc_in = nc.dram_tensor("bfc", [128, c.GB], F32, kind="ExternalInput")
    qidx_in = nc.dram_tensor("qidx", [1, c.T], F32, kind="ExternalInput")
    kofs_in = nc.dram_tensor("kofs", [128, c.KC], F32, kind="ExternalInput")
    y_out = nc.dram_tensor("y", [c.T, c.D], F32, kind="ExternalOutput")

    pairs = [[2 * b, 2 * b + 1] for b in range(c.B)]

    def bcast(dram, p=128):
        # partition-broadcast DMA source: read row 0 for every partition
        return bass.AP(tensor=dram, offset=0, ap=[[0, p], [1, dram.shape[1]]])

    with tile.TileContext(nc) as tc, ExitStack() as es:
        dpool = es.enter_context(tc.tile_pool(name="dram", bufs=1, space="DRAM"))
        gconst = es.enter_context(tc.tile_pool(name="gconst", bufs=1))

        # DRAM bounce buffers for the pairwise AllGathers (fp8), split into
        # head-halves so each AG launches as soon as its half is produced
        # and attention unblocks incrementally.
        HB = c.D // 2  # feature rows per head-half
        kb_loc = [dpool.tile([HB, c.T], F8, name=f"kb_loc{h}")
                  for h in range(2)]
        kb_full = [dpool.tile([2 * HB, c.T], F8, name=f"kb_full{h}")
                   for h in range(2)]
        vb_loc = [dpool.tile([c.T, HB], F8, name=f"vb_loc{h}")
                  for h in range(2)]
        vb_full = [dpool.tile([2 * c.T, HB], F8, name=f"vb_full{h}")
                   for h in range(2)]

        # ---------------- global constants ----------------
        ident_bf = gconst.tile([128, 128], BF16)
        make_identity(nc, ident_bf[:])
        eps_t = gconst.tile([128, 1], F32)
        nc.vector.memset(eps_t[:], EPS)
        ones1_bf = gconst.tile([1, 128], BF16)
        nc.vector.memset(ones1_bf[:], 1.0)

        def layernorm_t(src_tiles, wcol, bcol, out_tile, out_tag,
                        dout_tile=None):
            """token-major LN over free axis + transpose to feature-major.

            Writes DC slices of out_tile [128, DC, T]; the LN affine (w, b)
            is applied per-partition at the transpose eviction. If dout_tile
            is given, also writes the fp8 quantization residual
            (exact - out) for compensated-fp8 GEMMs."""
            with (
                tc.tile_pool(name=f"ln_{out_tag}", bufs=3) as lnp,
                tc.tile_pool(name=f"ln2_{out_tag}", bufs=4) as lnp2,
                tc.tile_pool(name=f"ps_tr_{out_tag}", bufs=4,
                             space="PSUM") as ps_tr,
            ):
                for tb in range(c.TB):
                    src = src_tiles[tb]
                    nsg = c.D // 512 if c.D % 512 == 0 else 1
                    sgw = c.D // nsg
                    st = lnp.tile([128, nsg, 6], F32, tag="st")
                    for sg in range(nsg):
                        nc.vector.bn_stats(
                            out=st[:, sg, :],
                            in_=src[:, sg * sgw:(sg + 1) * sgw])
                    mv = lnp.tile([128, 2], F32, tag="mv")
                    nc.vector.bn_aggr(out=mv[:], in_=st[:])
                    sd = lnp.tile([128, 1], F32, tag="sd")
                    nc.scalar.activation(sd[:], mv[:, 1:2], AF.Sqrt,
                                         bias=eps_t[:, 0:1])
                    rs = lnp.tile([128, 1], F32, tag="rs")
                    nc.vector.reciprocal(rs[:], sd[:])
                    ht_ = lnp.tile([128, c.D], BF16, tag="h")
                    # normalize split DVE/Pool, sized by their relative
                    # throughput, to shorten the per-tile critical chain
                    hD = 640
                    nc.vector.tensor_scalar(
                        out=ht_[:, 0:hD], in0=src[:, 0:hD],
                        scalar1=mv[:, 0:1], scalar2=rs[:, 0:1],
                        op0=ALU.subtract, op1=ALU.mult)
                    nc.gpsimd.tensor_scalar(
                        out=ht_[:, hD:c.D], in0=src[:, hD:c.D],
                        scalar1=mv[:, 0:1], scalar2=rs[:, 0:1],
                        op0=ALU.subtract, op1=ALU.mult)
                    for i in range(c.DC):
                        pt = ps_tr.tile([128, 128], BF16, tag="tr")
                        nc.tensor.transpose(
                            pt[:], ht_[:, i * 128:(i + 1) * 128], ident_bf[:])
                        # ACT evict with the LN affine folded in as
                        # per-partition scale/bias (Pool cannot read PSUM)
                        tsl = slice(tb * 128, (tb + 1) * 128)
                        nc.scalar.activation(
                            out_tile[:, i, tsl], pt[:],
                            AF.Identity, bias=bcol[:, i:i + 1],
                            scale=wcol[:, i:i + 1])
                        if dout_tile is not None:
                            tmp = lnp2.tile([128, 128], BF16, tag="tmp")
                            nc.vector.tensor_scalar(
                                out=tmp[:], in0=pt[:],
                                scalar1=wcol[:, i:i + 1],
                                scalar2=bcol[:, i:i + 1],
                                op0=ALU.mult, op1=ALU.add)
                            nc.vector.tensor_tensor(
                                out=dout_tile[:, i, tsl], in0=tmp[:],
                                in1=out_tile[:, i, tsl],
                                op=ALU.subtract)

        # ================= phase A: LN1 + QKV =================
        es_x = ExitStack()
        xpool = es_x.enter_context(tc.tile_pool(name="xpool", bufs=1, side="left"))
        xt = []
        for tb in range(c.TB):
            t = xpool.tile([128, c.D], F32, tag=f"x{tb}", name=f"x{tb}")
            # alternate DMA queues and fetch halves so LN stats (which work
            # on 512-wide subgroups) start as early as possible
            eng = nc.sync if tb % 2 == 0 else nc.gpsimd
            for hx in range(2):
                csl = slice(hx * 512, (hx + 1) * 512)
                eng.dma_start(out=t[:, csl],
                              in_=x_in[tb * 128:(tb + 1) * 128, csl])
            xt.append(t)

        es_qt = ExitStack()
        qtpool = es_qt.enter_context(tc.tile_pool(name="qtpool", bufs=1, side="right"))
        qtp = []
        for j in range(c.H // c.HPB):
            qtp.append(qtpool.tile([128, c.T], BF16, tag=f"qt{j}",
                                   name=f"qt{j}"))

        with (
            tc.tile_pool(name="aconst", bufs=1) as aconst,
            tc.tile_pool(name="htp", bufs=1) as htpool,
        ):
            ln1wc = aconst.tile([128, c.DC], F32)
            ln1bc = aconst.tile([128, c.DC], F32)
            for t, d in [(ln1wc, ln1wc_in), (ln1bc, ln1bc_in)]:
                nc.sync.dma_start(out=t[:], in_=d[:, :])
            battn_qk = aconst.tile([128, 2 * c.DC], F32)
            nc.sync.dma_start(out=battn_qk[:], in_=battn_qk_in[:, :])

            NP = c.DC // 2  # DoubleRow contraction pairs

            with (
                tc.tile_pool(name="wa", bufs=3) as wap,
                tc.tile_pool(name="kout", bufs=6) as kop,
                tc.tile_pool(name="ps_mm", bufs=4, space="PSUM") as psmm,
            ):
                # allocate + fetch the QKV weights BEFORE the LN pools so
                # their SBUF space doesn't alias LN transients (which would
                # make the DMA wait for LN1 to release buffers)
                wk_all = wap.tile([128, c.DC, c.D], F8, tag="wa",
                                  name="wk_all")
                nc.sync.dma_start(
                    out=wk_all[:],
                    in_=w_attn[:, c.D:2 * c.D].rearrange(
                        "(i p) f -> p i f", p=128))
                wv_all = wap.tile([128, c.DC, c.D], F8, tag="wa",
                                  name="wv_all")
                nc.sync.dma_start(
                    out=wv_all[:],
                    in_=w_attn[:, 2 * c.D:3 * c.D].rearrange(
                        "(i p) f -> p i f", p=128))

                ht = htpool.tile([128, c.DC, c.T], F8, name="ht_all")
                layernorm_t(xt, ln1wc, ln1bc, ht, "ht")

                # ---- K and V passes, interleaved by head-half so the
                # collective order is K0, V0, K1, V1 (attention for the
                # first head-half unblocks while the second half transfers)
                MH = c.DC // 2  # m-chunks per head-half
                for kh in range(2):
                    # k^T pass for this head-half (feature-major)
                    for ml in range(MH):
                        m = kh * MH + ml
                        for th in range(c.QH):
                            ps = psmm.tile([128, c.QF], F32, tag="ps")
                            for i in range(NP):
                                nc.tensor.matmul(
                                    ps[:], wk_all[:, 2 * i:2 * i + 2,
                                                  m * 128:(m + 1) * 128],
                                    ht[:, 2 * i:2 * i + 2,
                                       th * c.QF:(th + 1) * c.QF],
                                    start=(i == 0), stop=(i == NP - 1),
                                    perf_mode=DR)
                            ko = kop.tile([128, c.QF], F8, tag="ko")
                            # alternate ACT/DVE so the eviction tail on the
                            # path to the K AllGather halves
                            if (m + th) % 2 == 0:
                                nc.scalar.activation(
                                    ko[:], ps[:], AF.Identity,
                                    bias=battn_qk[:, c.DC + m:c.DC + m + 1],
                                    scale=1.0 / WS)
                            else:
                                nc.vector.tensor_scalar(
                                    out=ko[:], in0=ps[:], scalar1=1.0 / WS,
                                    scalar2=battn_qk[:, c.DC + m:
                                                     c.DC + m + 1],
                                    op0=ALU.mult, op1=ALU.add)
                            nc.sync.dma_start(
                                out=kb_loc[kh][ml * 128:(ml + 1) * 128,
                                               th * c.QF:(th + 1) * c.QF],
                                in_=ko[:])
                    nc.gpsimd.collective_compute(
                        "AllGather", ALU.bypass, ins=[kb_loc[kh][:]],
                        outs=[kb_full[kh][:]], replica_groups=pairs)
                    # v pass for this head-half (token-major)
                    for tb in range(c.TB):
                        ps = psmm.tile([128, c.VF], F32, tag="ps")
                        for i in range(NP):
                            nc.tensor.matmul(
                                ps[:], ht[:, 2 * i:2 * i + 2,
                                          tb * 128:(tb + 1) * 128],
                                wv_all[:, 2 * i:2 * i + 2,
                                       kh * c.VF:(kh + 1) * c.VF],
                                start=(i == 0), stop=(i == NP - 1),
                                perf_mode=DR)
                        vo = kop.tile([128, c.VF], F8, tag="vo")
                        nc.vector.tensor_scalar(
                            out=vo[:], in0=ps[:], scalar1=1.0 / WS,
                            scalar2=None, op0=ALU.mult)
                        nc.sync.dma_start(
                            out=vb_loc[kh][tb * 128:(tb + 1) * 128, :],
                            in_=vo[:])
                    nc.gpsimd.collective_compute(
                        "AllGather", ALU.bypass, ins=[vb_loc[kh][:]],
                        outs=[vb_full[kh][:]], replica_groups=pairs)

                # ---- q^T pass (feature-major, stays in SBUF) ----
                wq_all = wap.tile([128, c.DC, c.D], F8, tag="wa",
                                  name="wq_all")
                nc.sync.dma_start(
                    out=wq_all[:],
                    in_=w_attn[:, 0:c.D].rearrange(
                        "(i p) f -> p i f", p=128))
                for m in range(c.DC):
                    for th in range(c.QH):
                        ps = psmm.tile([128, c.QF], F32, tag="ps")
                        for i in range(NP):
                            nc.tensor.matmul(
                                ps[:], wq_all[:, 2 * i:2 * i + 2,
                                              m * 128:(m + 1) * 128],
                                ht[:, 2 * i:2 * i + 2,
                                   th * c.QF:(th + 1) * c.QF],
                                start=(i == 0), stop=(i == NP - 1),
                                perf_mode=DR)
                        # scale by 1/sqrt(HD) at eviction (bias pre-scaled)
                        nc.scalar.activation(
                            qtp[m][:, th * c.QF:(th + 1) * c.QF], ps[:],
                            AF.Identity, bias=battn_qk[:, m:m + 1],
                            scale=1.0 / (WS * math.sqrt(c.HD)))

        # ================= phase B: attention =================
        # prefetch c_proj weights during attention (scalar DMA ring)
        es_wc = ExitStack()
        wcp = es_wc.enter_context(tc.tile_pool(name="wc", bufs=1, side="left"))
        wc_all = wcp.tile([128, c.DC, c.D], BF16, tag="wc", name="wc_all")
        nc.scalar.dma_start(
            out=wc_all[:],
            in_=w_cproj[:, :].rearrange("(i p) f -> p i f", p=128))

        es_at = ExitStack()
        atpool = es_at.enter_context(tc.tile_pool(name="atpool", bufs=1, side="left"))
        at = []
        for j in range(c.DC):
            at.append(atpool.tile([128, c.T], BF16, tag=f"at{j}",
                                  name=f"at{j}"))

        with (
            tc.tile_pool(name="bconst", bufs=1) as bconst,
            tc.tile_pool(name="mask", bufs=1) as maskp,
            tc.tile_pool(name="kv", bufs=5) as kvp,
            tc.tile_pool(name="pt5", bufs=56) as ptp5,
            tc.tile_pool(name="pt2", bufs=32) as ptp2,
            tc.tile_pool(name="rec", bufs=6) as recp,
            tc.tile_pool(name="ps_s", bufs=3, space="PSUM") as pss,
            tc.tile_pool(name="ps_o", bufs=2, space="PSUM") as pso,
        ):
            qidx = bconst.tile([128, c.T], F32)
            nc.sync.dma_start(out=qidx[:], in_=bcast(qidx_in))
            kofs = bconst.tile([128, c.KC], F32)
            nc.sync.dma_start(out=kofs[:], in_=kofs_in[:, :])

            # per-slot chunk lists (compile-time causal structure)
            slot_chunks = []
            for sl in range(c.SLOTS):
                cl = [kc for kc in range(c.KC)
                      if chunk_absblk(c, kc) <= 2 * sl + 1]
                slot_chunks.append(cl)

            # group q-slots in pairs: one 512-wide QK/exp per k-chunk
            groups = []
            sl = 0
            while sl < c.SLOTS:
                g = [sl, sl + 1] if sl + 1 < c.SLOTS else [sl]
                groups.append(g)
                sl += len(g)

            # pre-generate boundary masks per (group, chunk) where the chunk
            # may cross the causal diagonal. For "full"-class boundary chunks
            # (absblk <= 2*g0+1) only the LOWER slot of the group can be
            # non-visible (the upper slot's stripes sit strictly after the
            # chunk), so every mask is one slot (BS) wide: full-class masks
            # cover slot g0's columns, diff-class masks cover slot g1's.
            masks = {}
            for gi, g in enumerate(groups):
                for kc in slot_chunks[g[-1]]:
                    ab = chunk_absblk(c, kc)
                    if ab < 2 * g[0]:
                        continue
                    msl_slot = g[0] if ab <= 2 * g[0] + 1 else g[-1]
                    qsl = slice(msl_slot * c.BS, (msl_slot + 1) * c.BS)
                    mk = maskp.tile([128, c.BS], BF16,
                                    tag=f"mk{gi}_{kc}",
                                    name=f"mk{gi}_{kc}")
                    nc.vector.tensor_scalar(
                        out=mk[:], in0=qidx[:, qsl],
                        scalar1=kofs[:, kc:kc + 1], scalar2=None,
                        op0=ALU.is_ge)
                    masks[(gi, kc)] = mk

            for jj in range(c.H // c.HPB):
                kh = jj // 4        # head-half buffer index
                jl = jj % 4
                ktp = kvp.tile([128, c.S], F8, tag="ktp")
                for hp in range(c.HPB):
                    hl = 2 * jl + hp  # head within the half
                    psl = slice(hp * 64, hp * 64 + 64)
                    nc.sync.dma_start(
                        out=ktp[psl, 0:c.T],
                        in_=kb_full[kh][64 * hl:64 * hl + 64, :])
                    nc.sync.dma_start(
                        out=ktp[psl, c.T:c.S],
                        in_=kb_full[kh][HB + 64 * hl:HB + 64 * hl + 64, :])
                # V for both heads of the pair, with an appended ones column
                # per head. Chunk row padded to 144 so the DoubleRow
                # Ldweights outer step is 16B-aligned (head slots at 0, 72).
                vt = kvp.tile([128, c.KC, 144], F8, tag="vt")
                vt4 = vt[:, :, :].rearrange("p kc (h f) -> p kc h f", f=72)
                for hp in range(c.HPB):
                    fb = 128 * jl + 64 * hp
                    nc.sync.dma_start(
                        out=vt[:, :, hp * 72:hp * 72 + 64],
                        in_=vb_full[kh][:, fb:fb + 64].rearrange(
                            "(kc p) f -> p kc f", p=128))
                nc.gpsimd.memset(vt4[:, :, :, 64:65], 1.0)

                for hp in range(c.HPB):
                    base = hp * 64
                    for gi, g in enumerate(groups):
                        gw = len(g) * c.BS
                        gq = slice(g[0] * c.BS, g[0] * c.BS + gw)
                        rhs_q = qtp[jj][base:base + 64, gq]
                        cl_all = slot_chunks[g[-1]]
                        full = [kc for kc in cl_all
                                if not (len(g) == 2 and
                                        chunk_absblk(c, kc) > 2 * g[0] + 1)]
                        diff = [kc for kc in cl_all if kc not in full]
                        pt_of = {}
                        nmask = 0
                        for plist, w, dtag in ((full, gw, False),
                                               (diff, c.BS, True)):
                            rq = (qtp[jj][base:base + 64,
                                          g[1] * c.BS:(g[1] + 1) * c.BS]
                                  if dtag else rhs_q)
                            for kc0, kc1 in pairup(plist):
                                ps = pss.tile([128, 2, gw], F32, tag="s")
                                for j, kc in ((0, kc0), (1, kc1)):
                                    nc.tensor.matmul(
                                        ps[:, j, 0:w],
                                        ktp[base:base + 64,
                                            kc * 128:(kc + 1) * 128],
                                        rq, start=True, stop=True)
                                ptpool = ptp5 if w == gw else ptp2
                                pt = ptpool.tile([128, 2, w], F8,
                                                 tag=f"pt{w}")
                                nc.scalar.activation(pt[:, :, :],
                                                     ps[:, :, 0:w], AF.Exp)
                                for j, kc in ((0, kc0), (1, kc1)):
                                    if (gi, kc) in masks:
                                        mw = masks[(gi, kc)]
                                        # full-class masks only touch the
                                        # lower slot's BS columns
                                        psl_ = pt[:, j, 0:c.BS]
                                        # split mask load DVE / Pool
                                        eng = (nc.vector if nmask % 2 == 0
                                               else nc.gpsimd)
                                        eng.tensor_mul(psl_, psl_, mw[:])
                                        nmask += 1
                                    pt_of[kc] = (pt, j, dtag)
                        for half, sl in enumerate(g):
                            qsl = slice(sl * c.BS, (sl + 1) * c.BS)
                            cl = slot_chunks[sl]
                            cpairs = pairup(cl)
                            po = pso.tile([65, c.BS], F32, tag="o")
                            for n, (kc0, kc1) in enumerate(cpairs):
                                pt, j0, dtag = pt_of[kc0]
                                assert pt_of[kc1][0] is pt and j0 == 0
                                col = 0 if dtag else half * c.BS
                                nc.tensor.matmul(
                                    po[:],
                                    vt[:, kc0:kc0 + 2,
                                       hp * 72:hp * 72 + 65],
                                    pt[:, :, col:col + c.BS],
                                    start=(n == 0),
                                    stop=(n == len(cpairs) - 1),
                                    perf_mode=DR)
                            # normalize by softmax denominator (row 64):
                            # reciprocal -> Pool partition-broadcast -> mul
                            rec = recp.tile([1, c.BS], F32, tag="rec")
                            with nc.allow_low_precision(
                                    reason="softmax denom reciprocal"):
                                nc.vector.reciprocal(rec[:], po[64:65, :])
                            bcr = recp.tile([64, c.BS], F32, tag="bcr")
                            nc.gpsimd.partition_broadcast(bcr[:], rec[:])
                            nc.vector.tensor_mul(
                                at[jj][base:base + 64, qsl], po[0:64, :],
                                bcr[:])

        es_qt.close()

        # ================= phase C: c_proj + residual =================
        es_x2 = ExitStack()
        x2pool = es_x2.enter_context(tc.tile_pool(name="x2pool", bufs=1, side="right"))
        x2t = []
        with (
            tc.tile_pool(name="cconst", bufs=1) as cconst,
            tc.tile_pool(name="ps_c", bufs=4, space="PSUM") as psc,
        ):
            bcp_r = cconst.tile([1, c.D], BF16)
            nc.sync.dma_start(out=bcp_r[:], in_=bcp_in[:, :])
            for tb in range(c.TB):
                x2 = x2pool.tile([128, c.D], F32, tag=f"x2_{tb}",
                                 name=f"x2_{tb}")
                for fh in range(c.FH):
                    fsl = slice(fh * c.VF, (fh + 1) * c.VF)
                    ps = psc.tile([128, c.VF], F32, tag="ps")
                    for i in range(c.DC):
                        nc.tensor.matmul(
                            ps[:], at[i][:, tb * 128:(tb + 1) * 128],
                            wc_all[:, i, fh * c.VF:(fh + 1) * c.VF],
                            start=(i == 0), stop=False)
                    # rank-1 bias add: ones^T @ b_cproj_eff
                    nc.tensor.matmul(ps[:], ones1_bf[:], bcp_r[0:1, fsl],
                                     start=False, stop=True)
                    nc.vector.tensor_add(x2[:, fsl], ps[:], xt[tb][:, fsl])
                x2t.append(x2)

        es_at.close()
        es_wc.close()
        es_x.close()

        # ================= phase D: LN2 + MLP =================
        with (
            tc.tile_pool(name="dconst", bufs=1) as dconst,
            tc.tile_pool(name="gt", bufs=1) as gtp,
            tc.tile_pool(name="wm", bufs=1) as wmp,
        ):
            ln2wc = dconst.tile([128, c.DC], F32)
            ln2bc = dconst.tile([128, c.DC], F32)
            for t, d in [(ln2wc, ln2wc_in), (ln2bc, ln2bc_in)]:
                nc.sync.dma_start(out=t[:], in_=d[:, :])
            bmp_r = dconst.tile([1, c.D], BF16)
            nc.sync.dma_start(out=bmp_r[:], in_=bmp_in[:, :])
            bfc = dconst.tile([128, c.GB], F32)
            nc.sync.dma_start(out=bfc[:], in_=bfc_in[:, :])

            # prefetch mproj weights early (scalar DMA queue, overlaps
            # cproj / LN2 / fc)
            wm_all, dwm_all = [], []
            for fh in range(c.FH):
                fsl = slice(fh * c.VF, (fh + 1) * c.VF)
                wm = wmp.tile([128, c.GB, c.VF], F8, tag=f"wm{fh}",
                              name=f"wm{fh}")
                nc.scalar.dma_start(
                    out=wm[:],
                    in_=w_mp8[:, fsl].rearrange(
                        "(g p) f -> p g f", p=128))
                wm_all.append(wm)
                dwm = wmp.tile([128, c.GB, c.VF], F8, tag=f"dwm{fh}",
                               name=f"dwm{fh}")
                nc.scalar.dma_start(
                    out=dwm[:],
                    in_=dw_mp8[:, fsl].rearrange(
                        "(g p) f -> p g f", p=128))
                dwm_all.append(dwm)

            GH = c.GB // 2  # mproj contraction halves
            NP = c.DC // 2

            def mm3(ps, wt, dwt, ht8, dht8, nw, wsl, hsl):
                """3-pass compensated-fp8 DoubleRow accumulation into ps:
                W8@h8 + W8@dh8 + dW8@h8 (all in the WS-scaled domain)."""
                passes = [(wt, ht8), (wt, dht8), (dwt, ht8)]
                for p, (wa, ha) in enumerate(passes):
                    for i in range(nw):
                        nc.tensor.matmul(
                            ps, wa[:, 2 * i:2 * i + 2, wsl],
                            ha[:, 2 * i:2 * i + 2, hsl],
                            start=(p == 0 and i == 0),
                            stop=(p == 2 and i == nw - 1),
                            perf_mode=DR)

            with (
                tc.tile_pool(name="mtp", bufs=1) as mtpool,
                tc.tile_pool(name="wf", bufs=2) as wfp,
            ):
                mt8 = mtpool.tile([128, c.DC, c.T], F8, name="mt8_all")
                dmt8 = mtpool.tile([128, c.DC, c.T], F8, name="dmt8_all")
                layernorm_t(x2t, ln2wc, ln2bc, mt8, "mt", dout_tile=dmt8)

                g8 = gtp.tile([128, c.GB, c.T], F8, name="g8_all")
                dg8 = gtp.tile([128, c.GB, c.T], F8, name="dg8_all")

                # ---------------- fc + gelu + mproj (lo half) ----------
                with (
                    tc.tile_pool(name="gtmp", bufs=4) as gtmpp,
                    tc.tile_pool(name="ps_g", bufs=4, space="PSUM") as psg,
                    tc.tile_pool(name="ps_m", bufs=4, space="PSUM") as psml,
                ):
                    GPW = 512 // 128  # g-blocks per W_fc slab
                    wf8_s = dwf8_s = None
                    for gb in range(c.GB):
                        if gb % GPW == 0:
                            j = gb // GPW
                            jsl = slice(j * 512, (j + 1) * 512)
                            wf8_s = wfp.tile([128, c.DC, 512], F8,
                                             tag="wf8", name=f"wf8_{gb}")
                            nc.scalar.dma_start(
                                out=wf8_s[:],
                                in_=w_fc8[:, jsl].rearrange(
                                    "(i p) f -> p i f", p=128))
                            dwf8_s = wfp.tile([128, c.DC, 512], F8,
                                              tag="dwf8", name=f"dwf8_{gb}")
                            nc.scalar.dma_start(
                                out=dwf8_s[:],
                                in_=dw_fc8[:, jsl].rearrange(
                                    "(i p) f -> p i f", p=128))
                        gl = (gb % GPW) * 128
                        for th in range(c.QH):
                            tsl = slice(th * c.QF, (th + 1) * c.QF)
                            ps = psg.tile([128, c.QF], F32, tag="ps")
                            mm3(ps[:], wf8_s, dwf8_s, mt8, dmt8, NP,
                                slice(gl, gl + 128), tsl)
                            gtmp = gtmpp.tile([128, c.QF], BF16, tag="gt")
                            nc.scalar.activation(
                                gtmp[:], ps[:], AF.Gelu_apprx_tanh,
                                bias=bfc[:, gb:gb + 1], scale=1.0 / WS)
                            nc.vector.tensor_copy(g8[:, gb, tsl], gtmp[:])
                            nc.vector.tensor_tensor(
                                out=dg8[:, gb, tsl], in0=gtmp[:],
                                in1=g8[:, gb, tsl], op=ALU.subtract)
                        if gb == GH - 1:
                            # low half of the mproj contraction starts while
                            # fc computes the high half; partial sums are
                            # accumulated into x2t (free after LN2)
                            for tb in range(c.TB):
                                tbs = slice(tb * 128, (tb + 1) * 128)
                                for fh in range(c.FH):
                                    fsl = slice(fh * c.VF, (fh + 1) * c.VF)
                                    ps = psml.tile([128, c.VF], F32,
                                                   tag="ps")
                                    mm3(ps[:], g8, dg8,
                                        wm_all[fh], dwm_all[fh], GH // 2,
                                        tbs, slice(0, c.VF))
                                    mtmp = gtmpp.tile([128, c.VF], F32,
                                                      tag="mtmp")
                                    nc.vector.tensor_scalar(
                                        out=mtmp[:], in0=ps[:],
                                        scalar1=1.0 / WS, scalar2=None,
                                        op0=ALU.mult)
                                    nc.vector.tensor_add(
                                        x2t[tb][:, fsl], x2t[tb][:, fsl],
                                        mtmp[:])

            # ---------------- mproj (hi half) + residual ----------------
            with (
                tc.tile_pool(name="yout", bufs=3) as yop,
                tc.tile_pool(name="ps_m2", bufs=4, space="PSUM") as psm,
            ):
                for tb in range(c.TB):
                    tbs = slice(tb * 128, (tb + 1) * 128)
                    yo = yop.tile([128, c.D], F32, tag="yo")
                    for fh in range(c.FH):
                        fsl = slice(fh * c.VF, (fh + 1) * c.VF)
                        ps = psm.tile([128, c.VF], F32, tag="ps")
                        for p, (ga, wa) in enumerate(
                                [(g8, wm_all[fh]), (dg8, wm_all[fh]),
                                 (g8, dwm_all[fh])]):
                            for i in range(GH // 2):
                                g = GH + 2 * i
                                nc.tensor.matmul(
                                    ps[:], ga[:, g:g + 2, tbs],
                                    wa[:, g:g + 2, :],
                                    start=(p == 0 and i == 0), stop=False,
                                    perf_mode=DR)
                        nc.tensor.matmul(ps[:], ones1_bf[:],
                                         bmp_r[0:1, fsl],
                                         start=False, stop=True)
                        yt = yop.tile([128, c.VF], F32, tag="yt")
                        nc.vector.tensor_scalar(
                            out=yt[:], in0=ps[:], scalar1=1.0 / WS,
                            scalar2=None, op0=ALU.mult)
                        nc.vector.tensor_add(yo[:, fsl], yt[:],
                                             x2t[tb][:, fsl])
                        # stream each feature-half out as soon as it's
                        # ready to shorten the final drain tail
                        nc.sync.dma_start(
                            out=y_out[tb * 128:(tb + 1) * 128, fsl],
                            in_=yo[:, fsl])

        es_x2.close()

    nc.compile()
    return nc


def make_core_inputs(cfg: Cfg, x, ln1_w, ln1_b, W_attn, b_attn, W_cproj,
                     b_cproj, ln2_w, ln2_b, W_fc, b_fc, W_mproj, b_mproj):
    """Split full inputs into one in_map per core."""
    c = cfg
    f32 = np.float32
    bf16 = ml_dtypes.bfloat16
    fp8 = ml_dtypes.float8_e4m3

    def lncol(v):
        return np.ascontiguousarray(
            np.asarray(v, f32).reshape(c.DC, 128).T)

    def comp8(W):
        """W*WS split into fp8 main + fp8 residual."""
        Ws = np.ascontiguousarray(np.asarray(W, f32) * WS)
        W8 = Ws.astype(fp8)
        dW8 = (Ws - W8.astype(f32)).astype(fp8)
        return W8, dW8

    b_v = np.asarray(b_attn[2 * c.D:3 * c.D], f32)
    bcp_eff = np.asarray(b_cproj, f32) + b_v @ np.asarray(W_cproj, f32)
    wf8, dwf8 = comp8(W_fc)
    wm8, dwm8 = comp8(W_mproj)
    shared = {
        "w_attn": np.ascontiguousarray(
            np.asarray(W_attn, f32) * WS).astype(fp8),
        "w_cproj": np.ascontiguousarray(W_cproj).astype(bf16),
        "w_fc8": wf8,
        "dw_fc8": dwf8,
        "w_mp8": wm8,
        "dw_mp8": dwm8,
        "ln1wc": lncol(ln1_w),
        "ln1bc": lncol(ln1_b),
        "ln2wc": lncol(ln2_w),
        "ln2bc": lncol(ln2_b),
        "bcp": np.ascontiguousarray(bcp_eff.reshape(1, c.D)).astype(bf16),
        # bmp lives in the WS-scaled mproj PSUM domain
        "bmp": np.ascontiguousarray(
            (np.asarray(b_mproj, f32) * WS).reshape(1, c.D)).astype(bf16),
        "bfc": np.ascontiguousarray(
            np.asarray(b_fc, f32).reshape(c.GB, 128).T),
    }
    bqk = np.asarray(b_attn[:2 * c.D], f32).reshape(2 * c.DC, 128).T.copy()
    bqk[:, :c.DC] *= 1.0 / math.sqrt(c.HD)
    shared["battn_qk"] = np.ascontiguousarray(bqk)

    in_maps = []
    for core in range(c.n_cores):
        b, half = core // 2, core % 2
        rows = core_rows(c, half)
        m = dict(shared)
        m["x"] = np.ascontiguousarray(np.asarray(x, f32)[b][rows])
        m["qidx"] = rows.astype(f32).reshape(1, c.T)
        kofs = np.empty((128, c.KC), f32)
        for kc in range(c.KC):
            parity = kc // c.KCH
            loc = (kc % c.KCH) * 128 + np.arange(128)
            kofs[:, kc] = (2 * (loc // c.BS) + parity) * c.BS + loc % c.BS
        m["kofs"] = kofs
        in_maps.append(m)
    return in_maps


def core_rows(cfg, half):
    """absolute sequence rows owned by a core with parity half"""
    c = cfg
    loc = np.arange(c.T)
    return (2 * (loc // c.BS) + half) * c.BS + loc % c.BS


_NC_CACHE = {}


def get_nc(cfg: Cfg):
    key = (cfg.B, cfg.S, cfg.D, cfg.H, cfg.F)
    if key not in _NC_CACHE:
        _NC_CACHE[key] = build(cfg)
    return _NC_CACHE[key]


def kernel(**inputs) -> np.ndarray:
    from concourse.bass_utils import run_bass_kernel_spmd

    cfg = Cfg()
    nc = get_nc(cfg)
    in_maps = make_core_inputs(cfg, **inputs)
    res = run_bass_kernel_spmd(nc, in_maps, core_ids=list(range(cfg.n_cores)))
    B, S, D, T = cfg.B, cfg.S, cfg.D, cfg.T
    out = np.empty((B, S, D), np.float32)
    for core in range(cfg.n_cores):
        b, half = core // 2, core % 2
        out[b, core_rows(cfg, half), :] = res.results[core]["y"]
    return out
